# revision 1
# baseline (speedup 1.0000x reference)
"""Trainium2 Bass kernel: reversible block with 4-group dilated attention.

Reference computation (per batch b, seq n=8192, d_model=1024, DM=512):
    x1, x2 = split(x, 2, -1)
    y1 = x1 + AttnBlock(LN(x2; ln1)) ; y2 = x2 + FFN(LN(y1; ln2))
    out = concat(y1, y2)
AttnBlock(z) = LN(DilatedAttn(zWq+bq, zWk+bk, zWv+bv); ln_attn) @ Wo + bo
DilatedAttn: heads split into 4 groups of 2; group i attends within segments
of length s_i in (1024,2048,4096,8192) at stride r_i in (1,2,4,8), offset i%r_i.

Sharding: 8 cores = batch(2) x seq-quarter(4).  Each quarter of 2048 tokens is
self-contained for groups 0/1; for groups 2/3 the (strided) key/value source
rows are replicated to each core (host-side slicing), so there is no
cross-core communication.  Everything on-device runs feature-major
([channel, token]) with bf16 matmul operands and fp32 accumulation/residuals.
"""

import numpy as np

import concourse.bass as bass
import concourse.mybir as mybir
import concourse.tile as tile
from concourse import bacc
from concourse.bass_utils import run_bass_kernel_spmd

F32 = mybir.dt.float32
BF16 = mybir.dt.bfloat16
AF = mybir.ActivationFunctionType
OP = mybir.AluOpType

DM = 512              # working width seen by attention/FFN
HD = 64               # head dim
T_LOC = 2048          # tokens per core
T_G2 = 1024           # group-2 replicated key rows (one 4096-segment, stride 4)
T_G3 = 1024           # group-3 replicated key rows (whole seq, stride 8)
NCH = DM // 128       # 4 channel chunks
EPS = 1e-5
SCALE = 1.0 / 8.0     # 1/sqrt(HD)

# order of [512]-vectors packed into the "vecs"/"vecsT" inputs
VEC_NAMES = ["bq", "bk", "bv", "bo", "b1", "b2",
             "ln1_g", "ln1_b", "lna_g", "lna_b", "ln2_g", "ln2_b"]
NVEC = 16  # padded columns

WEIGHT_NAMES = ["Wq", "Wk", "Wv", "Wo", "W1", "W2"]


def ts(i, size):
    return slice(i * size, (i + 1) * size)


def _emit(ctx, tc, P):
    """Emit the whole per-core program into TileContext tc."""
    nc = tc.nc

    # ---------------- pools ----------------
    # NOTE: within a pool, every distinct `tag` gets its own `bufs` slots
    # sized to the max tile of that tag.  SBUF budget is ~208KB/partition.
    pool = lambda **kw: ctx.enter_context(tc.tile_pool(**kw))
    consts = pool(name="consts", bufs=1)     # ~5KB: vectors, ones, bvb
    wtagp = pool(name="wtagp", bufs=12)      # 12KB: bf16 weight chunks (shared slots)
    stage = pool(name="stage", bufs=2)       # fp32 DMA staging
    xbp = pool(name="xbp", bufs=4)           # 16KB: bf16 casts of x2-side inputs
    x2np = pool(name="x2np", bufs=4)         # 16KB: LN1 outputs
    acts = pool(name="acts", bufs=12)        # 48KB: large bf16 activations
    vtp = pool(name="vtp", bufs=1)           # ~10KB: one consolidated V tile per group
    otp = pool(name="otp", bufs=1)           # 16KB: attention output (feature-major)
    sqp = pool(name="sqp", bufs=4)           # 3KB: squared tiles for stats
    rmrp = pool(name="rmrp", bufs=2)         # 16KB: R / MR stat rows (bf16, replicated)
    smp = pool(name="smp", bufs=2)           # ~24KB: small stat tiles
    ebp = pool(name="ebp", bufs=7)           # 10KB: exp(S) tiles
    yp = pool(name="yp", bufs=2)             # 8KB: fp32 output staging
    workp = pool(name="workp", bufs=3)       # 12KB: LN apply temps (shared tag)

    pmm = pool(name="pmm", bufs=4, space="PSUM")    # 4 banks: matmul outs + stats
    psb = pool(name="psb", bufs=2, space="PSUM")    # 4 banks: attention logits
    pst = pmm

    # ---------------- constants ----------------
    ones = consts.tile([128, 128], BF16, name="ones")
    nc.vector.memset(ones, 1.0)
    eps_col = consts.tile([128, 1], F32, name="eps_col")
    nc.vector.memset(eps_col, EPS)
    from concourse.masks import make_identity
    ident = consts.tile([128, 128], BF16, name="ident")
    make_identity(nc, ident[:])

    # weights -> bf16 tiles wb[name][k] : [128, DM].  Loaded lazily so the
    # shared wtag slots can recycle (Wq/Wk/Wv early, Wo/W1/W2 later).
    wb = {}

    def load_weight(w):
        if w in wb:
            return
        wb[w] = []
        for k in range(NCH):
            t = wtagp.tile([128, DM], BF16, tag="wtag", name=f"wb_{w}_{k}")
            nc.gpsimd.dma_start(out=t, in_=P[w][ts(k, 128), :])  # casting DMA
            wb[w].append(t)

    # per-channel vectors, channel-major: vst[k] = [128, NVEC]
    vst = []
    for k in range(NCH):
        t = consts.tile([128, NVEC], F32, name=f"vst_{k}")
        nc.sync.dma_start(out=t, in_=P["vecsT"][ts(k, 128), :])
        vst.append(t)

    def vcol(name, k):
        return vst[k][:, VEC_NAMES.index(name):VEC_NAMES.index(name) + 1]

    # bv broadcast across partitions (needed in token-major V tiles):
    # bvb[k] = [128, 128] where free dim = output channels k*128..(k+1)*128
    bvb = []
    bv_i = VEC_NAMES.index("bv")
    for k in range(NCH):
        t = consts.tile([128, 128], F32, name=f"bvb_{k}")
        nc.sync.dma_start(out=t, in_=P["vecs"][bv_i:bv_i + 1, ts(k, 128)].to_broadcast([128, 128]))
        bvb.append(t)

    # attention output assembly tiles, bf16 feature-major [512, T_LOC]
    oT = []
    for i in range(4):
        t = otp.tile([128, T_LOC], BF16, name=f"oT_{i}", tag=f"oT_{i}")
        oT.append(t)
        if i > 0:
            nc.vector.memset(t, 0.0)   # groups 1..3 leave gaps

    # ---------------- LN helpers (feature-major) ----------------
    def ln_alloc(name):
        R = rmrp.tile([128, T_LOC], BF16, tag="R", name=f"R_{name}")
        MR = rmrp.tile([128, T_LOC], BF16, tag="MR", name=f"MR_{name}")
        return R, MR

    def ln_stats_chunk(xb_tiles, tq, R, MR, name):
        """One 512-column chunk of feature-major LN stats.
        R = rstd rows (replicated over partitions), MR = mean*rstd."""
        s_ps = pmm.tile([128, 512], F32, tag="mm", name=f"sps_{name}_{tq}")
        q_ps = pmm.tile([128, 512], F32, tag="mm", name=f"qps_{name}_{tq}")
        for k in range(NCH):
            xsl = xb_tiles[k][:, ts(tq, 512)]
            sq = sqp.tile([128, 512], BF16, tag="sq", name=f"sq_{name}_{tq}_{k}")
            nc.vector.tensor_mul(out=sq, in0=xsl, in1=xsl)
            nc.tensor.matmul(s_ps, lhsT=ones, rhs=xsl, start=(k == 0), stop=(k == NCH - 1))
            nc.tensor.matmul(q_ps, lhsT=ones, rhs=sq, start=(k == 0), stop=(k == NCH - 1))
        m = smp.tile([128, 512], BF16, tag="m", name=f"m_{name}_{tq}")
        nc.scalar.activation(out=m, in_=s_ps, func=AF.Identity, scale=1.0 / DM)
        msq = smp.tile([128, 512], F32, tag="msq", name=f"msq_{name}_{tq}")
        nc.vector.tensor_mul(out=msq, in0=m, in1=m)
        var = smp.tile([128, 512], F32, tag="var", name=f"var_{name}_{tq}")
        nc.vector.scalar_tensor_tensor(out=var, in0=q_ps, scalar=1.0 / DM, in1=msq,
                                       op0=OP.mult, op1=OP.subtract)
        lnv = smp.tile([128, 512], F32, tag="lnv", name=f"lnv_{name}_{tq}")
        nc.scalar.activation(out=lnv, in_=var, func=AF.Ln, bias=eps_col)
        nc.scalar.activation(out=R[:, ts(tq, 512)], in_=lnv, func=AF.Exp, scale=-0.5)
        nc.vector.tensor_mul(out=MR[:, ts(tq, 512)], in0=m, in1=R[:, ts(tq, 512)])

    def ln_stats(xb_tiles, T, name):
        R, MR = ln_alloc(name)
        for tq in range(T // 512):
            ln_stats_chunk(xb_tiles, tq, R, MR, name)
        return R, MR

    def ln_apply_chunk(xb_tiles, R, MR, gname, bname, outs, name, lo, width):
        """out = (x*R - MR) * g + b over columns [lo, lo+width), all NCH chunks."""
        c = slice(lo, lo + width)
        for k in range(NCH):
            u = workp.tile([128, 1024], BF16, tag="uw", name=f"u_{name}_{k}_{lo}")
            nc.vector.tensor_mul(out=u[:, 0:width], in0=xb_tiles[k][:, c], in1=R[:, c])
            w = workp.tile([128, 1024], BF16, tag="uw", name=f"w_{name}_{k}_{lo}")
            nc.vector.tensor_sub(out=w[:, 0:width], in0=u[:, 0:width], in1=MR[:, c])
            nc.vector.tensor_scalar(out=outs[k][:, c], in0=w[:, 0:width],
                                    scalar1=vcol(gname, k), scalar2=vcol(bname, k),
                                    op0=OP.mult, op1=OP.add)

    def ln_apply(xb_tiles, R, MR, gname, bname, out_pool, T, name, out_tag=None):
        outs = [out_pool.tile([128, T], BF16, tag=(out_tag or "x2n"),
                              name=f"{name}_n_{k}") for k in range(NCH)]
        for lo in range(0, T, 1024):
            ln_apply_chunk(xb_tiles, R, MR, gname, bname, outs, name, lo,
                           min(1024, T - lo))
        return outs

    def load_and_cast(param, T, name):
        """DMA fp32 feature-major input, casting to bf16 in the SWDGE DMA."""
        xb = []
        for k in range(NCH):
            t = xbp.tile([128, T], BF16, tag="xb", name=f"xb_{name}_{k}")
            nc.gpsimd.dma_start(out=t, in_=param[ts(k, 128), :])
            xb.append(t)
        return xb

    def proj(dst, wname, bname, src_tiles, m, T, src_ap_fn=None):
        """dst [128, T] bf16 = (src @ W[:, m-block] + b) feature-major.
        src_ap_fn(tile, nq) may supply a (strided) token AP per 512-chunk."""
        w = min(512, T)
        for nq in range(T // w):
            ps = pmm.tile([128, 512], F32, tag="mm", name=f"ps_{wname}_{m}_{nq}")
            for k in range(NCH):
                rhs = (src_ap_fn(src_tiles[k], nq) if src_ap_fn
                       else src_tiles[k][:, ts(nq, w)])
                nc.tensor.matmul(ps[:, 0:w], lhsT=wb[wname][k][:, ts(m, 128)], rhs=rhs,
                                 start=(k == 0), stop=(k == NCH - 1))
            nc.scalar.activation(out=dst[:, ts(nq, w)], in_=ps[:, 0:w],
                                 func=AF.Identity, bias=vcol(bname, m))

    # token-major V tiles: [128 tokens, n_chunks, 2 heads, 65]
    # (65th col = 1.0 for the softmax denominator row of the AV matmul)
    def make_vt(src_tiles, tok_ap_fn, n_chunks, group):
        vt = vtp.tile([128, n_chunks, 2, 65], BF16, tag=f"vt{group}",
                      name=f"vt_{group}")
        nc.vector.memset(vt[:, :, :, 64:65], 1.0)
        for c in range(n_chunks):
            ps = pmm.tile([128, 128], F32, tag="mm", name=f"psv_{group}_{c}")
            for k in range(NCH):
                nc.tensor.matmul(ps, lhsT=tok_ap_fn(src_tiles[k], c),
                                 rhs=wb["Wv"][k][:, ts(group, 128)],
                                 start=(k == 0), stop=(k == NCH - 1))
            nc.vector.scalar_tensor_tensor(
                out=vt[:, c, :, 0:64],
                in0=ps[:].rearrange("p (h d) -> p h d", h=2),
                scalar=1.0,
                in1=bvb[group][:].rearrange("p (h d) -> p h d", h=2),
                op0=OP.bypass, op1=OP.add)
        return [vt[:, c] for c in range(n_chunks)]

    # ---------------- attention inner ----------------
    def do_attn(name, h, q_ap, o_ap, key_ap_fn, vt_fn, nq):
        kc_per_big = 1024 // nq
        ebs = []
        for big in range(8 // kc_per_big):
            sb = psb.tile([128, 1024], F32, tag="sbig", name=f"sb_{name}_{big}")
            for j in range(kc_per_big):
                kc = big * kc_per_big + j
                nc.tensor.matmul(sb[:, ts(j, nq)], lhsT=key_ap_fn(kc, h), rhs=q_ap,
                                 start=True, stop=True)
            eb = ebp.tile([128, 1024], BF16, tag="eb", name=f"eb_{name}_{big}")
            nc.scalar.activation(out=eb, in_=sb, func=AF.Exp, scale=SCALE)
            ebs.append(eb)
        ops = pmm.tile([128, 512], F32, tag="mm", name=f"ops_{name}")
        for kc in range(8):
            big, j = divmod(kc, kc_per_big)
            nc.tensor.matmul(ops[0:65, 0:nq], lhsT=vt_fn(kc)[:, h, 0:65],
                             rhs=ebs[big][:, ts(j, nq)],
                             start=(kc == 0), stop=(kc == 7))
        rc = smp.tile([1, 512], BF16, tag="rc", name=f"rc_{name}")
        with nc.allow_low_precision(reason="softmax denom reciprocal in bf16"):
            nc.vector.reciprocal(out=rc[:, 0:nq], in_=ops[64:65, 0:nq])
        # broadcast 1/denom across the 64 head dims via a K=1 outer product
        rb_ps = pmm.tile([128, 512], F32, tag="mm", name=f"rbps_{name}")
        nc.tensor.matmul(rb_ps[0:64, 0:nq], lhsT=ones[0:1, 0:64], rhs=rc[:, 0:nq],
                         start=True, stop=True)
        rb = smp.tile([64, 512], F32, tag="rb", name=f"rb_{name}")
        nc.vector.tensor_copy(out=rb[:, 0:nq], in_=rb_ps[0:64, 0:nq])
        nc.vector.tensor_mul(out=o_ap, in0=ops[0:64, 0:nq], in1=rb[:, 0:nq])

    hs = lambda h: slice(h * 64, (h + 1) * 64)

    def dil_ap(tile_, r, off):
        return tile_[:, :].rearrange("p (t r) -> p t r", r=r)[:, :, off]

    # ---------------- phase 1: local LN1 + Q ----------------
    xb_loc = load_and_cast(P["x2T"], T_LOC, "loc")
    R_loc, MR_loc = ln_stats(xb_loc, T_LOC, "loc")
    x2n = ln_apply(xb_loc, R_loc, MR_loc, "ln1_g", "ln1_b", x2np, T_LOC, "loc")

    # compact per-group Q: only the dilated query positions are projected
    q_sizes = [T_LOC, 1024, 512, 256]
    QT = [acts.tile([128, q_sizes[g]], BF16, tag="act5", name=f"QT_{g}")
          for g in range(4)]
    load_weight("Wq")
    proj(QT[0], "Wq", "bq", x2n, 0, T_LOC)
    proj(QT[1], "Wq", "bq", x2n, 1, 1024,
         lambda t, nq: dil_ap(t, 2, 1)[:, ts(nq, 512)])
    proj(QT[2], "Wq", "bq", x2n, 2, 512,
         lambda t, nq: dil_ap(t, 4, 2)[:, ts(nq, 512)])
    proj(QT[3], "Wq", "bq", x2n, 3, 256,
         lambda t, nq: dil_ap(t, 8, 3)[:, 0:256])

    # ---------------- phase 2: side groups 2/3 (chain + attention) ----------
    def side_group(pname, group, name):
        xbg = load_and_cast(P[pname], 1024, name)
        Rg, MRg = ln_stats(xbg, 1024, name)
        x2ng = ln_apply(xbg, Rg, MRg, "ln1_g", "ln1_b", x2np, 1024, name,
                        out_tag="x2n_side")
        load_weight("Wk")
        load_weight("Wv")
        ktg = acts.tile([128, 1024], BF16, tag="act5", name=f"KTg_{group}")
        proj(ktg, "Wk", "bk", x2ng, group, 1024)
        vtg = make_vt(x2ng, lambda t, c: t[:, ts(c, 128)], 8, group)
        return ktg, vtg

    KTg2, vt2 = side_group("xg2T", 2, "g2")
    for h in range(2):
        do_attn(f"g2_{h}", h, QT[2][hs(h), 0:512],
                dil_ap(oT[2], 4, 2)[hs(h), 0:512],
                lambda kc, h_: KTg2[hs(h_), ts(kc, 128)],
                lambda kc: vt2[kc], 512)

    KTg3, vt3 = side_group("xg3T", 3, "g3")
    for h in range(2):
        do_attn(f"g3_{h}", h, QT[3][hs(h), 0:256],
                dil_ap(oT[3], 8, 3)[hs(h), 0:256],
                lambda kc, h_: KTg3[hs(h_), ts(kc, 128)],
                lambda kc: vt3[kc], 256)

    # ---------------- phase 3: local K / V ----------------
    KT0 = acts.tile([128, T_LOC], BF16, tag="act5", name="KT_0")
    proj(KT0, "Wk", "bk", x2n, 0, T_LOC)
    KTg1 = acts.tile([128, 1024], BF16, tag="act5", name="KT_1")
    proj(KTg1, "Wk", "bk", x2n, 1, 1024,
         lambda t, nq: dil_ap(t, 2, 1)[:, ts(nq, 512)])

    vt0 = make_vt(x2n, lambda t, c: t[:, ts(c, 128)], 16, 0)
    vt1 = make_vt(x2n, lambda t, c: dil_ap(t, 2, 1)[:, ts(c, 128)], 8, 1)

    # ------------- phase 4: local attention + LN_attn stats (interleaved) ----
    load_weight("Wo")
    R_a, MR_a = ln_alloc("lna")

    def g0_attn(seg, h, qc):
        do_attn(f"g0_{seg}_{h}_{qc}", h,
                QT[0][hs(h), seg * 1024 + qc * 512: seg * 1024 + (qc + 1) * 512],
                oT[0][hs(h), seg * 1024 + qc * 512: seg * 1024 + (qc + 1) * 512],
                lambda kc, h_: KT0[hs(h_), seg * 1024 + kc * 128: seg * 1024 + (kc + 1) * 128],
                lambda kc: vt0[seg * 8 + kc], 512)

    def g1_attn(h, qc):
        do_attn(f"g1_{h}_{qc}", h, QT[1][hs(h), ts(qc, 512)],
                dil_ap(oT[1], 2, 1)[hs(h), ts(qc, 512)],
                lambda kc, h_: KTg1[hs(h_), ts(kc, 128)],
                lambda kc: vt1[kc], 512)

    # Phase 5 halves are interleaved into local attention: the LN_attn
    # stats/apply and Wo projection for columns [0,1024) run (on PE/DVE)
    # while the second half of attention keeps ACT busy with exps.
    o_n = [acts.tile([128, T_LOC], BF16, tag="act5", name=f"lna_n_{k}")
           for k in range(NCH)]
    y1b = [acts.tile([128, T_LOC], BF16, tag="act5", name=f"y1b_{m}")
           for m in range(4)]
    load_weight("W1")
    R_2, MR_2 = ln_alloc("ln2")

    def phase5_half(half):
        ln_apply_chunk(oT, R_a, MR_a, "lna_g", "lna_b", o_n, "lna", half * 1024, 1024)
        for nq in (2 * half, 2 * half + 1):
            for m in range(4):
                ps = pmm.tile([128, 512], F32, tag="mm", name=f"pso_{m}_{nq}")
                for k in range(NCH):
                    nc.tensor.matmul(ps, lhsT=wb["Wo"][k][:, ts(m, 128)],
                                     rhs=o_n[k][:, ts(nq, 512)],
                                     start=(k == 0), stop=False)
                x1f = stage.tile([128, 512], BF16, tag="x1stage", name=f"x1f_{m}_{nq}")
                nc.gpsimd.dma_start(out=x1f, in_=P["x1T"][ts(m, 128), ts(nq, 512)])
                nc.tensor.matmul(ps, lhsT=ident, rhs=x1f, start=False, stop=True)
                y1f = yp.tile([128, 512], F32, tag="y1f", name=f"y1f_{m}_{nq}")
                nc.scalar.activation(out=y1f, in_=ps, func=AF.Identity,
                                     bias=vcol("bo", m))
                nc.sync.dma_start(out=P["out"][ts(m, 128), ts(nq, 512)], in_=y1f)
                nc.scalar.activation(out=y1b[m][:, ts(nq, 512)], in_=y1f,
                                     func=AF.Identity)
        for tq in (2 * half, 2 * half + 1):
            ln_stats_chunk(y1b, tq, R_2, MR_2, "ln2")

    # columns 0:1024 of local attention, then their LN_attn stats + phase 5,
    # overlapped with the second half of attention.
    for qc in range(2):
        for h in range(2):
            g0_attn(0, h, qc)
    for h in range(2):
        g1_attn(h, 0)
    ln_stats_chunk(oT, 0, R_a, MR_a, "lna")
    ln_stats_chunk(oT, 1, R_a, MR_a, "lna")
    phase5_half(0)
    for qc in range(2):
        for h in range(2):
            g0_attn(1, h, qc)
    for h in range(2):
        g1_attn(h, 1)
    ln_stats_chunk(oT, 2, R_a, MR_a, "lna")
    ln_stats_chunk(oT, 3, R_a, MR_a, "lna")
    phase5_half(1)

    # ---------------- phase 6: LN2 + FFN + residual, pipelined ---------------
    y1n = [acts.tile([128, T_LOC], BF16, tag="act5", name=f"ln2_n_{k}")
           for k in range(NCH)]
    for half in range(2):
        ln_apply_chunk(y1b, R_2, MR_2, "ln2_g", "ln2_b", y1n, "ln2", half * 1024, 1024)
    hb = [acts.tile([128, T_LOC], BF16, tag="act5", name=f"hb_{m}")
          for m in range(4)]
    load_weight("W2")
    for half in range(2):
        for nq in (2 * half, 2 * half + 1):
            for m in range(4):
                ps = pmm.tile([128, 512], F32, tag="mm", name=f"psh_{m}_{nq}")
                for k in range(NCH):
                    nc.tensor.matmul(ps, lhsT=wb["W1"][k][:, ts(m, 128)],
                                     rhs=y1n[k][:, ts(nq, 512)],
                                     start=(k == 0), stop=(k == NCH - 1))
                nc.scalar.activation(out=hb[m][:, ts(nq, 512)], in_=ps, func=AF.Relu,
                                     bias=vcol("b1", m))
        for nq in (2 * half, 2 * half + 1):
            for m in range(4):
                ps = pmm.tile([128, 512], F32, tag="mm", name=f"psy2_{m}_{nq}")
                for k in range(NCH):
                    nc.tensor.matmul(ps, lhsT=wb["W2"][k][:, ts(m, 128)],
                                     rhs=hb[k][:, ts(nq, 512)],
                                     start=(k == 0), stop=False)
                x2f = stage.tile([128, 512], BF16, tag="x2stage", name=f"x2f_{m}_{nq}")
                nc.gpsimd.dma_start(out=x2f, in_=P["x2T"][ts(m, 128), ts(nq, 512)])
                nc.tensor.matmul(ps, lhsT=ident, rhs=x2f, start=False, stop=True)
                y2f = yp.tile([128, 512], F32, tag="y2f", name=f"y2f_{m}_{nq}")
                nc.scalar.activation(out=y2f, in_=ps, func=AF.Identity,
                                     bias=vcol("b2", m))
                nc.sync.dma_start(out=P["out"][ts(m + 4, 128), ts(nq, 512)], in_=y2f)


_ACT_PATCHED = False


def _patch_act_tables():
    """Make every activation func we use resolve to the one table set that
    contains them all (natural_log_exp_and_others), so the act-table-load
    pass emits a single load instead of thrashing Ln<->Exp (~1.3us each).
    Set ids are dict-order — membership is edited, order preserved."""
    global _ACT_PATCHED
    if _ACT_PATCHED:
        return
    _ACT_PATCHED = True
    import concourse.hw_specs as _hw
    _orig = _hw.get_activation_tables
    mine = {AF.Exp, AF.Ln, AF.Relu, AF.Identity, AF.Copy}

    def _patched(arch):
        t = _orig(arch)
        return {name: (s if name == "natural_log_exp_and_others" else s - mine)
                for name, s in t.items()}

    bacc.get_activation_tables = _patched


def build_nc():
    _patch_act_tables()
    nc = bacc.Bacc()
    P = {}
    P["x1T"] = nc.declare_dram_parameter("x1T", [DM, T_LOC], F32, isOutput=False)
    P["x2T"] = nc.declare_dram_parameter("x2T", [DM, T_LOC], F32, isOutput=False)
    P["xg2T"] = nc.declare_dram_parameter("xg2T", [DM, T_G2], F32, isOutput=False)
    P["xg3T"] = nc.declare_dram_parameter("xg3T", [DM, T_G3], F32, isOutput=False)
    for w in WEIGHT_NAMES:
        P[w] = nc.declare_dram_parameter(w, [DM, DM], F32, isOutput=False)
    P["vecsT"] = nc.declare_dram_parameter("vecsT", [DM, NVEC], F32, isOutput=False)
    P["vecs"] = nc.declare_dram_parameter("vecs", [NVEC, DM], F32, isOutput=False)
    P["out"] = nc.declare_dram_parameter("out", [2 * DM, T_LOC], F32, isOutput=True)

    from contextlib import ExitStack
    with tile.TileContext(nc) as tc:
        with ExitStack() as ctx:
            _emit(ctx, tc, P)
    nc.finalize()
    return nc


def make_in_maps(x, Wq, bq, Wk, bk, Wv, bv, ln_attn_g, ln_attn_b, Wo, bo,
                 W1, b1, W2, b2, ln1_g, ln1_b, ln2_g, ln2_b):
    """Shard the full inputs into 8 per-core input maps."""
    x = np.ascontiguousarray(np.asarray(x, dtype=np.float32))
    f32 = lambda a: np.ascontiguousarray(np.asarray(a, dtype=np.float32))
    Ws = {"Wq": f32(Wq), "Wk": f32(Wk), "Wv": f32(Wv),
          "Wo": f32(Wo), "W1": f32(W1), "W2": f32(W2)}
    vec_vals = [bq, bk, bv, bo, b1, b2, ln1_g, ln1_b, ln_attn_g, ln_attn_b,
                ln2_g, ln2_b]
    vecs = np.zeros((NVEC, DM), np.float32)
    for i, v in enumerate(vec_vals):
        vecs[i] = np.asarray(v, dtype=np.float32)
    vecsT = np.ascontiguousarray(vecs.T)

    in_maps = []
    for core in range(8):
        b, qt = divmod(core, 4)
        t0 = qt * T_LOC
        x1 = x[b, t0:t0 + T_LOC, 0:DM]
        x2 = x[b, t0:t0 + T_LOC, DM:2 * DM]
        seg2 = qt // 2
        xg2 = x[b, seg2 * 4096 + 2: (seg2 + 1) * 4096: 4, DM:2 * DM]
        xg3 = x[b, 3::8, DM:2 * DM]
        m = {
            "x1T": np.ascontiguousarray(x1.T),
            "x2T": np.ascontiguousarray(x2.T),
            "xg2T": np.ascontiguousarray(xg2.T),
            "xg3T": np.ascontiguousarray(xg3.T),
            "vecsT": vecsT,
            "vecs": vecs,
        }
        m.update(Ws)
        in_maps.append(m)
    return in_maps


_NC_CACHE = None


def get_nc():
    global _NC_CACHE
    if _NC_CACHE is None:
        _NC_CACHE = build_nc()
    return _NC_CACHE


def _execute(in_maps, trace=False, **kwargs):
    nc = get_nc()
    return run_bass_kernel_spmd(nc, in_maps, core_ids=list(range(8)),
                                trace=trace, **kwargs)


def assemble_output(results):
    out = np.empty((2, 8192, 2 * DM), np.float32)
    for core in range(8):
        b, qt = divmod(core, 4)
        out[b, qt * T_LOC:(qt + 1) * T_LOC, :] = results[core]["out"].T
    return out


def kernel(**inputs):
    in_maps = make_in_maps(**inputs)
    res = _execute(in_maps)
    return assemble_output(res.results)



# revision 38
# speedup vs baseline: 1.0363x; 1.0363x over previous
"""Trainium2 Bass kernel: reversible block with 4-group dilated attention.

Reference computation (per batch b, seq n=8192, d_model=1024, DM=512):
    x1, x2 = split(x, 2, -1)
    y1 = x1 + AttnBlock(LN(x2; ln1)) ; y2 = x2 + FFN(LN(y1; ln2))
    out = concat(y1, y2)
AttnBlock(z) = LN(DilatedAttn(zWq+bq, zWk+bk, zWv+bv); ln_attn) @ Wo + bo
DilatedAttn: heads split into 4 groups of 2; group i attends within segments
of length s_i in (1024,2048,4096,8192) at stride r_i in (1,2,4,8), offset i%r_i.

Sharding: 8 cores = batch(2) x seq-quarter(4).  Each quarter of 2048 tokens is
self-contained for groups 0/1; for groups 2/3 the (strided) key/value source
rows are replicated to each core (host-side slicing), so there is no
cross-core communication.  Everything on-device runs feature-major
([channel, token]) with bf16 matmul operands and fp32 accumulation/residuals.
"""

import numpy as np

import concourse.bass as bass
import concourse.mybir as mybir
import concourse.tile as tile
from concourse import bacc
from concourse.bass_utils import run_bass_kernel_spmd

F32 = mybir.dt.float32
BF16 = mybir.dt.bfloat16
AF = mybir.ActivationFunctionType
OP = mybir.AluOpType

DM = 512              # working width seen by attention/FFN
HD = 64               # head dim
T_LOC = 2048          # tokens per core
T_G2 = 1024           # group-2 replicated key rows (one 4096-segment, stride 4)
T_G3 = 1024           # group-3 replicated key rows (whole seq, stride 8)
NCH = DM // 128       # 4 channel chunks
EPS = 1e-5
SCALE = 1.0 / 8.0     # 1/sqrt(HD)

# order of [512]-vectors packed into the "vecs"/"vecsT" inputs
VEC_NAMES = ["bq", "bk", "bv", "bo", "b1", "b2",
             "ln1_g", "ln1_b", "lna_g", "lna_b", "ln2_g", "ln2_b"]
NVEC = 16  # padded columns

WEIGHT_NAMES = ["Wq", "Wk", "Wv", "Wo", "W1", "W2"]


def ts(i, size):
    return slice(i * size, (i + 1) * size)


def _emit(ctx, tc, P):
    """Emit the whole per-core program into TileContext tc."""
    nc = tc.nc

    # ---------------- pools ----------------
    # NOTE: within a pool, every distinct `tag` gets its own `bufs` slots
    # sized to the max tile of that tag.  SBUF budget is ~208KB/partition.
    pool = lambda **kw: ctx.enter_context(tc.tile_pool(**kw))
    consts = pool(name="consts", bufs=1)     # ~5KB: vectors, ones, bvb
    wtagp = pool(name="wtagp", bufs=12)      # 12KB: bf16 weight chunks (shared slots)
    stage = pool(name="stage", bufs=2)       # fp32 DMA staging
    xbp = pool(name="xbp", bufs=4)           # 16KB: bf16 casts of x2-side inputs
    x2np = pool(name="x2np", bufs=4)         # 16KB: LN1 outputs
    acts = pool(name="acts", bufs=12)        # 48KB: large bf16 activations
    vtp = pool(name="vtp", bufs=1)           # ~10KB: one consolidated V tile per group
    otp = pool(name="otp", bufs=1)           # 16KB: attention output (feature-major)
    sqp = pool(name="sqp", bufs=4)           # 3KB: squared tiles for stats
    rmrp = pool(name="rmrp", bufs=2)         # 16KB: R / MR stat rows (bf16, replicated)
    smp = pool(name="smp", bufs=2)           # ~24KB: small stat tiles
    ebp = pool(name="ebp", bufs=6)           # 12KB: exp(S) tiles
    yp = pool(name="yp", bufs=2)             # 8KB: fp32 output staging
    workp = pool(name="workp", bufs=3)       # 6KB: LN apply temps (shared tag)
    ffn1p = pool(name="ffn1p", bufs=4)       # 8KB: per-half LN2 outputs
    ffn2p = pool(name="ffn2p", bufs=4)       # 8KB: per-half ReLU outputs

    pmm = pool(name="pmm", bufs=4, space="PSUM")    # 4 banks: matmul outs + stats
    psb = pool(name="psb", bufs=2, space="PSUM")    # 4 banks: attention logits
    pst = pmm

    # ---------------- constants ----------------
    ones = consts.tile([128, 128], BF16, name="ones")
    nc.vector.memset(ones, 1.0)
    eps_col = consts.tile([128, 1], F32, name="eps_col")
    nc.vector.memset(eps_col, EPS)
    from concourse.masks import make_identity
    ident = consts.tile([128, 128], BF16, name="ident")
    make_identity(nc, ident[:])

    # weights -> bf16 tiles wb[name][k] : [128, DM].  Loaded lazily so the
    # shared wtag slots can recycle (Wq/Wk/Wv early, Wo/W1/W2 later).
    wb = {}

    def load_weight(w):
        if w in wb:
            return
        wb[w] = []
        for k in range(NCH):
            t = wtagp.tile([128, DM], BF16, tag="wtag", name=f"wb_{w}_{k}")
            nc.gpsimd.dma_start(out=t, in_=P[w][ts(k, 128), :])  # casting DMA
            wb[w].append(t)

    # per-channel vectors, channel-major: vst[k] = [128, NVEC]
    vst = []
    for k in range(NCH):
        t = consts.tile([128, NVEC], F32, name=f"vst_{k}")
        nc.sync.dma_start(out=t, in_=P["vecsT"][ts(k, 128), :])
        vst.append(t)

    def vcol(name, k):
        return vst[k][:, VEC_NAMES.index(name):VEC_NAMES.index(name) + 1]

    # bv broadcast across partitions (needed in token-major V tiles):
    # bvb[k] = [128, 128] where free dim = output channels k*128..(k+1)*128
    bvb = []
    bv_i = VEC_NAMES.index("bv")
    for k in range(NCH):
        t = consts.tile([128, 128], F32, name=f"bvb_{k}")
        nc.sync.dma_start(out=t, in_=P["vecs"][bv_i:bv_i + 1, ts(k, 128)].to_broadcast([128, 128]))
        bvb.append(t)

    # attention output assembly tiles, bf16 feature-major [512, T_LOC];
    # groups 1..3 need zeroed gaps — memsets are emitted later (low priority)
    # so they fill DVE slack instead of competing with the LN1 frontend.
    oT = [otp.tile([128, T_LOC], BF16, name=f"oT_{i}", tag=f"oT_{i}")
          for i in range(4)]
    for i in range(1, 4):
        nc.vector.memset(oT[i], 0.0)

    # ---------------- LN helpers (feature-major) ----------------
    # LN output is produced as just (x - m) * R.  gamma is pre-folded into
    # the projection weights on the host (W' = g*W) and beta into the
    # projection biases (b' = b + W^T ln_b), so every consumer matmul sees
    # the full LN for free and the apply is two cheap bf16 TensorTensors.
    def ln_alloc(name):
        R = rmrp.tile([128, T_LOC], BF16, tag="R", name=f"R_{name}")
        M = rmrp.tile([128, T_LOC], BF16, tag="M", name=f"M_{name}")
        return R, M

    def ln_stats_chunk(xb_tiles, tq, R, M, name, sq_act=False):
        """One 512-column chunk of feature-major LN stats.
        R = rstd rows, M = mean rows (both replicated over partitions).
        sq_act: square on ACT (frontend, where DVE is the bottleneck)."""
        s_ps = pmm.tile([128, 512], F32, tag="mm", name=f"sps_{name}_{tq}")
        q_ps = pmm.tile([128, 512], F32, tag="mm", name=f"qps_{name}_{tq}")
        for k in range(NCH):
            xsl = xb_tiles[k][:, ts(tq, 512)]
            sq = sqp.tile([128, 512], BF16, tag="sq", name=f"sq_{name}_{tq}_{k}")
            if sq_act:
                nc.scalar.activation(out=sq, in_=xsl, func=AF.Square)
            else:
                nc.vector.tensor_mul(out=sq, in0=xsl, in1=xsl)
            nc.tensor.matmul(s_ps, lhsT=ones, rhs=xsl, start=(k == 0), stop=(k == NCH - 1))
            nc.tensor.matmul(q_ps, lhsT=ones, rhs=sq, start=(k == 0), stop=(k == NCH - 1))
        m_sl = M[:, ts(tq, 512)]
        nc.scalar.activation(out=m_sl, in_=s_ps, func=AF.Identity, scale=1.0 / DM)
        msq = smp.tile([128, 512], BF16, tag="msq", name=f"msq_{name}_{tq}")
        nc.vector.tensor_mul(out=msq, in0=m_sl, in1=m_sl)
        var = smp.tile([128, 512], F32, tag="var", name=f"var_{name}_{tq}")
        nc.vector.scalar_tensor_tensor(out=var, in0=q_ps, scalar=1.0 / DM, in1=msq,
                                       op0=OP.mult, op1=OP.subtract)
        lnv = smp.tile([128, 512], F32, tag="lnv", name=f"lnv_{name}_{tq}")
        nc.scalar.activation(out=lnv, in_=var, func=AF.Ln, bias=eps_col)
        nc.scalar.activation(out=R[:, ts(tq, 512)], in_=lnv, func=AF.Exp, scale=-0.5)

    def ln_stats(xb_tiles, T, name, sq_act=False):
        R, M = ln_alloc(name)
        for tq in range(T // 512):
            ln_stats_chunk(xb_tiles, tq, R, M, name, sq_act=sq_act)
        return R, M

    def ln_apply_chunk(xb_tiles, R, M, outs, name, lo, width):
        """out = (x - m) * R over columns [lo, lo+width), all NCH chunks."""
        c = slice(lo, lo + width)
        for k in range(NCH):
            v = workp.tile([128, 1024], BF16, tag="uw", name=f"v_{name}_{k}_{lo}")
            nc.vector.tensor_sub(out=v[:, 0:width], in0=xb_tiles[k][:, c], in1=M[:, c])
            nc.vector.tensor_mul(out=outs[k][:, c], in0=v[:, 0:width], in1=R[:, c])

    def ln_apply(xb_tiles, R, M, out_pool, T, name, out_tag=None):
        outs = [out_pool.tile([128, T], BF16, tag=(out_tag or "x2n"),
                              name=f"{name}_n_{k}") for k in range(NCH)]
        for lo in range(0, T, 1024):
            ln_apply_chunk(xb_tiles, R, M, outs, name, lo, min(1024, T - lo))
        return outs

    def load_and_cast(param, T, name):
        """DMA fp32 feature-major input, casting to bf16 in the SWDGE DMA."""
        xb = []
        for k in range(NCH):
            t = xbp.tile([128, T], BF16, tag="xb", name=f"xb_{name}_{k}")
            nc.gpsimd.dma_start(out=t, in_=param[ts(k, 128), :])
            xb.append(t)
        return xb

    def proj(dst, wname, bname, src_tiles, m, T, src_ap_fn=None, nq_range=None):
        """dst [128, T] bf16 = (src @ W[:, m-block] + b) feature-major.
        src_ap_fn(tile, nq) may supply a (strided) token AP per 512-chunk.
        PSUM eviction + bias add on the Pool engine (ACT is the attention
        bottleneck, Pool has slack)."""
        w = min(512, T)
        for nq in (range(T // w) if nq_range is None else nq_range):
            ps = pmm.tile([128, 512], F32, tag="mm", name=f"ps_{wname}_{m}_{nq}")
            for k in range(NCH):
                rhs = (src_ap_fn(src_tiles[k], nq) if src_ap_fn
                       else src_tiles[k][:, ts(nq, w)])
                nc.tensor.matmul(ps[:, 0:w], lhsT=wb[wname][k][:, ts(m, 128)], rhs=rhs,
                                 start=(k == 0), stop=(k == NCH - 1))
            nc.scalar.activation(out=dst[:, ts(nq, w)], in_=ps[:, 0:w],
                                 func=AF.Identity, bias=vcol(bname, m))

    # token-major V tiles: [128 tokens, n_chunks, 2 heads, 65]
    # (65th col = 1.0 for the softmax denominator row of the AV matmul)
    # PSUM eviction + bias add runs on the idle Pool engine.
    def mk_vt(n_chunks, group):
        vt = vtp.tile([128, n_chunks, 2, 65], BF16, tag=f"vt{group}",
                      name=f"vt_{group}")
        nc.vector.memset(vt[:, :, :, 64:65], 1.0)
        return vt

    def fill_vt(vt, src_tiles, tok_ap_fn, group, chunks):
        for c in chunks:
            ps = pmm.tile([128, 128], F32, tag="mm", name=f"psv_{group}_{c}")
            for k in range(NCH):
                nc.tensor.matmul(ps, lhsT=tok_ap_fn(src_tiles[k], c),
                                 rhs=wb["Wv"][k][:, ts(group, 128)],
                                 start=(k == 0), stop=(k == NCH - 1))
            nc.vector.scalar_tensor_tensor(
                out=vt[:, c, :, 0:64],
                in0=ps[:].rearrange("p (h d) -> p h d", h=2),
                scalar=1.0,
                in1=bvb[group][:].rearrange("p (h d) -> p h d", h=2),
                op0=OP.bypass, op1=OP.add)

    def make_vt(src_tiles, tok_ap_fn, n_chunks, group):
        vt = mk_vt(n_chunks, group)
        fill_vt(vt, src_tiles, tok_ap_fn, group, range(n_chunks))
        return vt

    # ---------------- attention inner ----------------
    def do_attn(name, h, q_ap, o_ap, key_ap_fn, vt_fn, nq):
        kc_per_big = 1024 // nq
        ebs = []
        for big in range(8 // kc_per_big):
            sb = psb.tile([128, 1024], F32, tag="sbig", name=f"sb_{name}_{big}")
            for j in range(kc_per_big):
                kc = big * kc_per_big + j
                nc.tensor.matmul(sb[:, ts(j, nq)], lhsT=key_ap_fn(kc, h), rhs=q_ap,
                                 start=True, stop=True)
            eb = ebp.tile([128, 1024], BF16, tag="eb", name=f"eb_{name}_{big}")
            nc.scalar.activation(out=eb, in_=sb, func=AF.Exp, scale=SCALE)
            ebs.append(eb)
        ops = pmm.tile([128, 512], F32, tag="mm", name=f"ops_{name}")
        for kc in range(8):
            big, j = divmod(kc, kc_per_big)
            nc.tensor.matmul(ops[0:65, 0:nq], lhsT=vt_fn(kc)[:, h, 0:65],
                             rhs=ebs[big][:, ts(j, nq)],
                             start=(kc == 0), stop=(kc == 7))
        rc = smp.tile([1, 512], BF16, tag="rc", name=f"rc_{name}")
        with nc.allow_low_precision(reason="softmax denom reciprocal in bf16"):
            nc.vector.reciprocal(out=rc[:, 0:nq], in_=ops[64:65, 0:nq])
        # broadcast 1/denom across the 64 head dims on the idle Pool engine
        rb = smp.tile([64, 512], BF16, tag="rb", name=f"rb_{name}")
        nc.gpsimd.partition_broadcast(rb[:, 0:nq], rc[0:1, 0:nq], 64)
        nc.vector.tensor_mul(out=o_ap, in0=ops[0:64, 0:nq], in1=rb[:, 0:nq])

    hs = lambda h: slice(h * 64, (h + 1) * 64)

    def dil_ap(tile_, r, off):
        return tile_[:, :].rearrange("p (t r) -> p t r", r=r)[:, :, off]

    # -------- phase 1: LN1 half 0 -> seg-0 Q/K/V -> seg-0 attention --------
    # Getting attention (ACT exps + PE) started ~15us in, overlapped with the
    # DVE-heavy remainder of the LN1 frontend.
    xb_loc = load_and_cast(P["x2T"], T_LOC, "loc")
    R_loc, M_loc = ln_alloc("loc")
    ln_stats_chunk(xb_loc, 0, R_loc, M_loc, "loc")
    ln_stats_chunk(xb_loc, 1, R_loc, M_loc, "loc")
    ln_stats_chunk(xb_loc, 2, R_loc, M_loc, "loc", sq_act=True)
    ln_stats_chunk(xb_loc, 3, R_loc, M_loc, "loc", sq_act=True)
    x2n = [x2np.tile([128, T_LOC], BF16, tag="x2n", name=f"loc_n_{k}")
           for k in range(NCH)]
    ln_apply_chunk(xb_loc, R_loc, M_loc, x2n, "loc", 0, 1024)

    q_sizes = [T_LOC, 1024, 512, 256]
    QT = [acts.tile([128, q_sizes[g]], BF16, tag="act5", name=f"QT_{g}")
          for g in range(4)]
    load_weight("Wq")
    proj(QT[0], "Wq", "bq", x2n, 0, T_LOC, nq_range=(0, 1))
    load_weight("Wk")
    KT0 = acts.tile([128, T_LOC], BF16, tag="act5", name="KT_0")
    proj(KT0, "Wk", "bk", x2n, 0, T_LOC, nq_range=(0, 1))
    load_weight("Wv")
    vt0 = mk_vt(16, 0)
    fill_vt(vt0, x2n, lambda t, c: t[:, ts(c, 128)], 0, range(8))

    def g0_attn(seg, h, qc):
        do_attn(f"g0_{seg}_{h}_{qc}", h,
                QT[0][hs(h), seg * 1024 + qc * 512: seg * 1024 + (qc + 1) * 512],
                oT[0][hs(h), seg * 1024 + qc * 512: seg * 1024 + (qc + 1) * 512],
                lambda kc, h_: KT0[hs(h_), seg * 1024 + kc * 128: seg * 1024 + (kc + 1) * 128],
                lambda kc: vt0[:, seg * 8 + kc], 512)

    for qc in range(2):
        for h in range(2):
            g0_attn(0, h, qc)

    # -------- phase 2: LN1 half 1 + remaining local Q/K/V --------
    ln_apply_chunk(xb_loc, R_loc, M_loc, x2n, "loc", 1024, 1024)
    proj(QT[0], "Wq", "bq", x2n, 0, T_LOC, nq_range=(2, 3))
    proj(QT[1], "Wq", "bq", x2n, 1, 1024,
         lambda t, nq: dil_ap(t, 2, 1)[:, ts(nq, 512)])
    proj(QT[2], "Wq", "bq", x2n, 2, 512,
         lambda t, nq: dil_ap(t, 4, 2)[:, ts(nq, 512)])
    proj(QT[3], "Wq", "bq", x2n, 3, 256,
         lambda t, nq: dil_ap(t, 8, 3)[:, 0:256])
    proj(KT0, "Wk", "bk", x2n, 0, T_LOC, nq_range=(2, 3))
    KTg1 = acts.tile([128, 1024], BF16, tag="act5", name="KT_1")
    proj(KTg1, "Wk", "bk", x2n, 1, 1024,
         lambda t, nq: dil_ap(t, 2, 1)[:, ts(nq, 512)])
    fill_vt(vt0, x2n, lambda t, c: t[:, ts(c, 128)], 0, range(8, 16))
    vt1 = make_vt(x2n, lambda t, c: dil_ap(t, 2, 1)[:, ts(c, 128)], 8, 1)

    # -------- phase 3: side groups 2/3 (chain + attention) --------
    def side_group(xbg, group, name):
        Rg, Mg = ln_stats(xbg, 1024, name, sq_act=True)
        x2ng = ln_apply(xbg, Rg, Mg, x2np, 1024, name,
                        out_tag="x2n_side")
        ktg = acts.tile([128, 1024], BF16, tag="act5", name=f"KTg_{group}")
        proj(ktg, "Wk", "bk", x2ng, group, 1024)
        vtg = make_vt(x2ng, lambda t, c: t[:, ts(c, 128)], 8, group)
        return ktg, vtg

    xb_g2 = load_and_cast(P["xg2T"], 1024, "g2")
    KTg2, vt2 = side_group(xb_g2, 2, "g2")
    for h in range(2):
        do_attn(f"g2_{h}", h, QT[2][hs(h), 0:512],
                dil_ap(oT[2], 4, 2)[hs(h), 0:512],
                lambda kc, h_: KTg2[hs(h_), ts(kc, 128)],
                lambda kc: vt2[:, kc], 512)

    xb_g3 = load_and_cast(P["xg3T"], 1024, "g3")
    KTg3, vt3 = side_group(xb_g3, 3, "g3")
    for h in range(2):
        do_attn(f"g3_{h}", h, QT[3][hs(h), 0:256],
                dil_ap(oT[3], 8, 3)[hs(h), 0:256],
                lambda kc, h_: KTg3[hs(h_), ts(kc, 128)],
                lambda kc: vt3[:, kc], 256)

    # ------------- phase 4: local attention + LN_attn stats (interleaved) ----
    load_weight("Wo")
    R_a, M_a = ln_alloc("lna")

    def g1_attn(h, qc):
        do_attn(f"g1_{h}_{qc}", h, QT[1][hs(h), ts(qc, 512)],
                dil_ap(oT[1], 2, 1)[hs(h), ts(qc, 512)],
                lambda kc, h_: KTg1[hs(h_), ts(kc, 128)],
                lambda kc: vt1[:, kc], 512)

    # Phase 5 halves are interleaved into local attention: the LN_attn
    # stats/apply and Wo projection for columns [0,1024) run (on PE/DVE)
    # while the second half of attention keeps ACT busy with exps.
    o_n = [acts.tile([128, T_LOC], BF16, tag="act5", name=f"lna_n_{k}")
           for k in range(NCH)]
    y1b = [acts.tile([128, T_LOC], BF16, tag="act5", name=f"y1b_{m}")
           for m in range(4)]
    load_weight("W1")
    R_2, M_2 = ln_alloc("ln2")

    def phase5_half(half):
        ln_apply_chunk(oT, R_a, M_a, o_n, "lna", half * 1024, 1024)
        for nq in (2 * half, 2 * half + 1):
            for m in range(4):
                ps = pmm.tile([128, 512], F32, tag="mm", name=f"pso_{m}_{nq}")
                for k in range(NCH):
                    nc.tensor.matmul(ps, lhsT=wb["Wo"][k][:, ts(m, 128)],
                                     rhs=o_n[k][:, ts(nq, 512)],
                                     start=(k == 0), stop=False)
                x1f = stage.tile([128, 512], BF16, tag="x1stage", name=f"x1f_{m}_{nq}")
                nc.gpsimd.dma_start(out=x1f, in_=P["x1T"][ts(m, 128), ts(nq, 512)])
                nc.tensor.matmul(ps, lhsT=ident, rhs=x1f, start=False, stop=True)
                nc.scalar.activation(out=y1b[m][:, ts(nq, 512)], in_=ps,
                                     func=AF.Identity, bias=vcol("bo", m))
                nc.gpsimd.dma_start(out=P["out"][ts(m, 128), ts(nq, 512)],
                                    in_=y1b[m][:, ts(nq, 512)])
        for tq in (2 * half, 2 * half + 1):
            ln_stats_chunk(y1b, tq, R_2, M_2, "ln2", sq_act=True)

    # ---------------- phase 6: LN2 + FFN + residual, per-half ---------------
    # Per-half tiles live in dedicated pools so their allocation never waits
    # on slots held by still-running attention work (deadlock + overlap).
    def phase6_half(half):
        c = slice(half * 1024, half * 1024 + 1024)
        y1n_h = [ffn1p.tile([128, 1024], BF16, tag="y1n", name=f"ln2n_{half}_{k}")
                 for k in range(NCH)]
        for k in range(NCH):
            v = workp.tile([128, 1024], BF16, tag="uw", name=f"v_ln2_{half}_{k}")
            nc.vector.tensor_sub(out=v, in0=y1b[k][:, c], in1=M_2[:, c])
            nc.vector.tensor_mul(out=y1n_h[k], in0=v, in1=R_2[:, c])
        hb_h = [ffn2p.tile([128, 1024], BF16, tag="hb", name=f"hb_{half}_{m}")
                for m in range(4)]
        load_weight("W2")
        for j in range(2):
            nq = 2 * half + j
            for m in range(4):
                ps = pmm.tile([128, 512], F32, tag="mm", name=f"psh_{m}_{nq}")
                for k in range(NCH):
                    nc.tensor.matmul(ps, lhsT=wb["W1"][k][:, ts(m, 128)],
                                     rhs=y1n_h[k][:, ts(j, 512)],
                                     start=(k == 0), stop=(k == NCH - 1))
                nc.scalar.activation(out=hb_h[m][:, ts(j, 512)], in_=ps, func=AF.Relu,
                                     bias=vcol("b1", m))
        for j in range(2):
            nq = 2 * half + j
            for m in range(4):
                ps = pmm.tile([128, 512], F32, tag="mm", name=f"psy2_{m}_{nq}")
                for k in range(NCH):
                    nc.tensor.matmul(ps, lhsT=wb["W2"][k][:, ts(m, 128)],
                                     rhs=hb_h[k][:, ts(j, 512)],
                                     start=(k == 0), stop=False)
                x2f = stage.tile([128, 512], BF16, tag="x2stage", name=f"x2f_{m}_{nq}")
                nc.gpsimd.dma_start(out=x2f, in_=P["x2T"][ts(m, 128), ts(nq, 512)])
                nc.tensor.matmul(ps, lhsT=ident, rhs=x2f, start=False, stop=True)
                y2f = yp.tile([128, 512], F32, tag="y2f", name=f"y2f_{m}_{nq}")
                nc.scalar.activation(out=y2f, in_=ps, func=AF.Identity,
                                     bias=vcol("b2", m))
                nc.sync.dma_start(out=P["out"][ts(m + 4, 128), ts(nq, 512)], in_=y2f)

    # columns 0:1024 of local attention, then their LN_attn stats + phase 5/6,
    # overlapped with the second half of attention.
    for h in range(2):
        g1_attn(h, 0)
    ln_stats_chunk(oT, 0, R_a, M_a, "lna", sq_act=True)
    ln_stats_chunk(oT, 1, R_a, M_a, "lna", sq_act=True)
    for qc in range(2):
        for h in range(2):
            g0_attn(1, h, qc)
    for h in range(2):
        g1_attn(h, 1)
    ln_stats_chunk(oT, 2, R_a, M_a, "lna", sq_act=True)
    ln_stats_chunk(oT, 3, R_a, M_a, "lna", sq_act=True)
    phase5_half(0)
    phase6_half(0)
    phase5_half(1)
    phase6_half(1)


_ACT_PATCHED = False


def _patch_act_tables():
    """Make every activation func we use resolve to the one table set that
    contains them all (natural_log_exp_and_others), so the act-table-load
    pass emits a single load instead of thrashing Ln<->Exp (~1.3us each).
    Set ids are dict-order — membership is edited, order preserved."""
    global _ACT_PATCHED
    if _ACT_PATCHED:
        return
    _ACT_PATCHED = True
    import concourse.hw_specs as _hw
    _orig = _hw.get_activation_tables
    mine = {AF.Exp, AF.Ln, AF.Relu, AF.Identity, AF.Copy}

    def _patched(arch):
        t = _orig(arch)
        return {name: (s if name == "natural_log_exp_and_others" else s - mine)
                for name, s in t.items()}

    bacc.get_activation_tables = _patched


def build_nc():
    _patch_act_tables()
    nc = bacc.Bacc()
    P = {}
    P["x1T"] = nc.declare_dram_parameter("x1T", [DM, T_LOC], F32, isOutput=False)
    P["x2T"] = nc.declare_dram_parameter("x2T", [DM, T_LOC], F32, isOutput=False)
    P["xg2T"] = nc.declare_dram_parameter("xg2T", [DM, T_G2], F32, isOutput=False)
    P["xg3T"] = nc.declare_dram_parameter("xg3T", [DM, T_G3], F32, isOutput=False)
    for w in WEIGHT_NAMES:
        P[w] = nc.declare_dram_parameter(w, [DM, DM], F32, isOutput=False)
    P["vecsT"] = nc.declare_dram_parameter("vecsT", [DM, NVEC], F32, isOutput=False)
    P["vecs"] = nc.declare_dram_parameter("vecs", [NVEC, DM], F32, isOutput=False)
    P["out"] = nc.declare_dram_parameter("out", [2 * DM, T_LOC], F32, isOutput=True)

    from contextlib import ExitStack
    with tile.TileContext(nc) as tc:
        with ExitStack() as ctx:
            _emit(ctx, tc, P)
    nc.finalize()
    return nc


def make_in_maps(x, Wq, bq, Wk, bk, Wv, bv, ln_attn_g, ln_attn_b, Wo, bo,
                 W1, b1, W2, b2, ln1_g, ln1_b, ln2_g, ln2_b):
    """Shard the full inputs into 8 per-core input maps.

    LN gamma/beta are folded into the consumer projections here (pure host
    numpy): W' = g*W and b' = b + W^T ln_b, so the device LN apply is just
    (x - mean) * rstd."""
    x = np.ascontiguousarray(np.asarray(x, dtype=np.float32))
    f32 = lambda a: np.asarray(a, dtype=np.float32)
    Wq, Wk, Wv, Wo, W1, W2 = (f32(W) for W in (Wq, Wk, Wv, Wo, W1, W2))
    bq, bk, bv, bo, b1, b2 = (f32(b) for b in (bq, bk, bv, bo, b1, b2))
    g1, b1n = f32(ln1_g), f32(ln1_b)
    ga, ban = f32(ln_attn_g), f32(ln_attn_b)
    g2, b2n = f32(ln2_g), f32(ln2_b)
    cc = np.ascontiguousarray
    Ws = {"Wq": cc(g1[:, None] * Wq), "Wk": cc(g1[:, None] * Wk),
          "Wv": cc(g1[:, None] * Wv), "Wo": cc(ga[:, None] * Wo),
          "W1": cc(g2[:, None] * W1), "W2": cc(W2)}
    bq = bq + Wq.T @ b1n
    bk = bk + Wk.T @ b1n
    bv = bv + Wv.T @ b1n
    bo = bo + Wo.T @ ban
    b1 = b1 + W1.T @ b2n
    vec_vals = [bq, bk, bv, bo, b1, b2, ln1_g, ln1_b, ln_attn_g, ln_attn_b,
                ln2_g, ln2_b]
    vecs = np.zeros((NVEC, DM), np.float32)
    for i, v in enumerate(vec_vals):
        vecs[i] = np.asarray(v, dtype=np.float32)
    vecsT = np.ascontiguousarray(vecs.T)

    in_maps = []
    for core in range(8):
        b, qt = divmod(core, 4)
        t0 = qt * T_LOC
        x1 = x[b, t0:t0 + T_LOC, 0:DM]
        x2 = x[b, t0:t0 + T_LOC, DM:2 * DM]
        seg2 = qt // 2
        xg2 = x[b, seg2 * 4096 + 2: (seg2 + 1) * 4096: 4, DM:2 * DM]
        xg3 = x[b, 3::8, DM:2 * DM]
        m = {
            "x1T": np.ascontiguousarray(x1.T),
            "x2T": np.ascontiguousarray(x2.T),
            "xg2T": np.ascontiguousarray(xg2.T),
            "xg3T": np.ascontiguousarray(xg3.T),
            "vecsT": vecsT,
            "vecs": vecs,
        }
        m.update(Ws)
        in_maps.append(m)
    return in_maps


_NC_CACHE = None


def get_nc():
    global _NC_CACHE
    if _NC_CACHE is None:
        _NC_CACHE = build_nc()
    return _NC_CACHE


def _execute(in_maps, trace=False, **kwargs):
    nc = get_nc()
    return run_bass_kernel_spmd(nc, in_maps, core_ids=list(range(8)),
                                trace=trace, **kwargs)


def assemble_output(results):
    out = np.empty((2, 8192, 2 * DM), np.float32)
    for core in range(8):
        b, qt = divmod(core, 4)
        out[b, qt * T_LOC:(qt + 1) * T_LOC, :] = results[core]["out"].T
    return out


def kernel(**inputs):
    in_maps = make_in_maps(**inputs)
    res = _execute(in_maps)
    return assemble_output(res.results)



# revision 51
# speedup vs baseline: 1.0743x; 1.0366x over previous
"""Trainium2 Bass kernel: reversible block with 4-group dilated attention.

Reference computation (per batch b, seq n=8192, d_model=1024, DM=512):
    x1, x2 = split(x, 2, -1)
    y1 = x1 + AttnBlock(LN(x2; ln1)) ; y2 = x2 + FFN(LN(y1; ln2))
    out = concat(y1, y2)
AttnBlock(z) = LN(DilatedAttn(zWq+bq, zWk+bk, zWv+bv); ln_attn) @ Wo + bo
DilatedAttn: heads split into 4 groups of 2; group i attends within segments
of length s_i in (1024,2048,4096,8192) at stride r_i in (1,2,4,8), offset i%r_i.

Sharding: 8 cores = batch(2) x seq-quarter(4).  Each quarter of 2048 tokens is
self-contained for groups 0/1; for groups 2/3 the (strided) key/value source
rows are replicated to each core (host-side slicing), so there is no
cross-core communication.  Everything on-device runs feature-major
([channel, token]) with bf16 matmul operands and fp32 accumulation/residuals.
"""

import numpy as np

import concourse.bass as bass
import concourse.mybir as mybir
import concourse.tile as tile
from concourse import bacc
from concourse.bass_utils import run_bass_kernel_spmd

F32 = mybir.dt.float32
BF16 = mybir.dt.bfloat16
AF = mybir.ActivationFunctionType
OP = mybir.AluOpType

DM = 512              # working width seen by attention/FFN
HD = 64               # head dim
T_LOC = 2048          # tokens per core
T_G2 = 1024           # group-2 replicated key rows (one 4096-segment, stride 4)
T_G3 = 1024           # group-3 replicated key rows (whole seq, stride 8)
NCH = DM // 128       # 4 channel chunks
EPS = 1e-5
SCALE = 1.0 / 8.0     # 1/sqrt(HD)

# order of [512]-vectors packed into the "vecs"/"vecsT" inputs
VEC_NAMES = ["bq", "bk", "bv", "bo", "b1", "b2",
             "ln1_g", "ln1_b", "lna_g", "lna_b", "ln2_g", "ln2_b",
             "wkbar", "wvbar", "wobar"]
NVEC = 16  # padded columns

WEIGHT_NAMES = ["Wq", "Wk", "Wv", "Wo", "W1", "W2"]


def ts(i, size):
    return slice(i * size, (i + 1) * size)


def _emit(ctx, tc, P):
    """Emit the whole per-core program into TileContext tc."""
    nc = tc.nc

    # ---------------- pools ----------------
    # NOTE: within a pool, every distinct `tag` gets its own `bufs` slots
    # sized to the max tile of that tag.  SBUF budget is ~208KB/partition.
    pool = lambda **kw: ctx.enter_context(tc.tile_pool(**kw))
    consts = pool(name="consts", bufs=1)     # ~5KB: vectors, ones, bvb
    wtagp = pool(name="wtagp", bufs=12)      # 12KB: bf16 weight chunks (shared slots)
    stage = pool(name="stage", bufs=2)       # bf16 residual staging
    xbp = pool(name="xbp", bufs=4)           # 16KB: bf16 casts of x2-side inputs
    x2np = pool(name="x2np", bufs=4)         # 16KB: LN1 outputs
    acts = pool(name="acts", bufs=12)        # 48KB: large bf16 activations
    vtp = pool(name="vtp", bufs=1)           # ~10KB: one consolidated V tile per group
    otp = pool(name="otp", bufs=1)           # 16KB: attention output (feature-major)
    sqp = pool(name="sqp", bufs=4)           # 4KB: squared tiles for stats
    rmrp = pool(name="rmrp", bufs=2)         # 16KB: R / MR stat rows (bf16, replicated)
    smp = pool(name="smp", bufs=2)           # ~24KB: small stat tiles
    ebp = pool(name="ebp", bufs=6)           # 12KB: exp(S) tiles
    yp = pool(name="yp", bufs=2)             # 8KB: fp32 output staging
    workp = pool(name="workp", bufs=3)       # 6KB: LN apply temps (shared tag)
    ffn1p = pool(name="ffn1p", bufs=4)       # 8KB: per-half LN2 outputs
    ffn2p = pool(name="ffn2p", bufs=4)       # 8KB: per-half ReLU outputs

    pmm = pool(name="pmm", bufs=4, space="PSUM")    # 4 banks: matmul outs + stats
    psb = pool(name="psb", bufs=2, space="PSUM")    # 4 banks: attention logits
    pst = pmm

    # ---------------- constants ----------------
    ones = consts.tile([128, 128], BF16, name="ones")
    nc.vector.memset(ones, 1.0)
    eps_col = consts.tile([128, 1], F32, name="eps_col")
    nc.vector.memset(eps_col, EPS)
    from concourse.masks import make_identity
    ident = consts.tile([128, 128], BF16, name="ident")
    make_identity(nc, ident[:])

    # weights -> bf16 tiles wb[name][k] : [128, DM].  Loaded lazily so the
    # shared wtag slots can recycle (Wq/Wk/Wv early, Wo/W1/W2 later).
    wb = {}

    def load_weight(w):
        if w in wb:
            return
        wb[w] = []
        for k in range(NCH):
            t = wtagp.tile([128, DM], BF16, tag="wtag", name=f"wb_{w}_{k}")
            nc.gpsimd.dma_start(out=t, in_=P[w][ts(k, 128), :])  # casting DMA
            wb[w].append(t)

    # per-channel vectors, channel-major: vst[k] = [128, NVEC]
    vst = []
    for k in range(NCH):
        t = consts.tile([128, NVEC], F32, name=f"vst_{k}")
        nc.sync.dma_start(out=t, in_=P["vecsT"][ts(k, 128), :])
        vst.append(t)

    def vcol(name, k):
        return vst[k][:, VEC_NAMES.index(name):VEC_NAMES.index(name) + 1]

    # bv broadcast across partitions (needed in token-major V tiles):
    # bvb[k] = [128, 128] where free dim = output channels k*128..(k+1)*128
    bvb = []
    bv_i = VEC_NAMES.index("bv")
    for k in range(NCH):
        t = consts.tile([128, 128], F32, name=f"bvb_{k}")
        nc.sync.dma_start(out=t, in_=P["vecs"][bv_i:bv_i + 1, ts(k, 128)].to_broadcast([128, 128]))
        bvb.append(t)

    # attention output assembly tiles, bf16 feature-major [512, T_LOC];
    # groups 1..3 need zeroed gaps — memsets are emitted later (low priority)
    # so they fill DVE slack instead of competing with the LN1 frontend.
    oT = [otp.tile([128, T_LOC], BF16, name=f"oT_{i}", tag=f"oT_{i}")
          for i in range(4)]
    for i in range(1, 4):
        nc.vector.memset(oT[i], 0.0)

    # ---------------- LN helpers (feature-major) ----------------
    # LN output is produced as just (x - m) * R.  gamma is pre-folded into
    # the projection weights on the host (W' = g*W) and beta into the
    # projection biases (b' = b + W^T ln_b), so every consumer matmul sees
    # the full LN for free and the apply is two cheap bf16 TensorTensors.
    def ln_alloc(name):
        R = rmrp.tile([128, T_LOC], BF16, tag="R", name=f"R_{name}")
        M = rmrp.tile([128, T_LOC], BF16, tag="M", name=f"M_{name}")
        return R, M

    def ln_stats_chunk(xb_tiles, tq, R, M, name, sq_act=False):
        """One 512-column chunk of feature-major LN stats.
        R = rstd rows, M = mean rows (both replicated over partitions).
        sq_act: square on ACT (frontend, where DVE is the bottleneck)."""
        s_ps = pmm.tile([128, 512], F32, tag="mm", name=f"sps_{name}_{tq}")
        q_ps = pmm.tile([128, 512], F32, tag="mm", name=f"qps_{name}_{tq}")
        for k in range(NCH):
            xsl = xb_tiles[k][:, ts(tq, 512)]
            sq = sqp.tile([128, 512], BF16, tag="sq", name=f"sq_{name}_{tq}_{k}")
            if sq_act:
                nc.scalar.activation(out=sq, in_=xsl, func=AF.Square)
            else:
                nc.vector.tensor_mul(out=sq, in0=xsl, in1=xsl)
            nc.tensor.matmul(s_ps, lhsT=ones, rhs=xsl, start=(k == 0), stop=(k == NCH - 1))
            nc.tensor.matmul(q_ps, lhsT=ones, rhs=sq, start=(k == 0), stop=(k == NCH - 1))
        m_sl = M[:, ts(tq, 512)]
        nc.vector.tensor_scalar(out=m_sl, in0=s_ps, scalar1=1.0 / DM, scalar2=0.0,
                                op0=OP.mult, op1=OP.add)
        msq = smp.tile([128, 512], BF16, tag="msq", name=f"msq_{name}_{tq}")
        nc.vector.tensor_mul(out=msq, in0=m_sl, in1=m_sl)
        var = smp.tile([128, 512], F32, tag="var", name=f"var_{name}_{tq}")
        nc.vector.scalar_tensor_tensor(out=var, in0=q_ps, scalar=1.0 / DM, in1=msq,
                                       op0=OP.mult, op1=OP.subtract)
        lnv = smp.tile([128, 512], F32, tag="lnv", name=f"lnv_{name}_{tq}")
        nc.scalar.activation(out=lnv, in_=var, func=AF.Ln, bias=eps_col)
        nc.scalar.activation(out=R[:, ts(tq, 512)], in_=lnv, func=AF.Exp, scale=-0.5)

    def ln_stats(xb_tiles, T, name, sq_act=False):
        R, M = ln_alloc(name)
        for tq in range(T // 512):
            ln_stats_chunk(xb_tiles, tq, R, M, name, sq_act=sq_act)
        return R, M

    def ln_apply_chunk(xb_tiles, R, M, outs, name, lo, width, mul_only=False):
        """out = (x - m) * R over columns [lo, lo+width), all NCH chunks.
        mul_only: out = x * R; the mean term is restored by a rank-1
        -wbar (x) (m*r) matmul inside the consumer projection."""
        c = slice(lo, lo + width)
        for k in range(NCH):
            if mul_only:
                nc.vector.tensor_mul(out=outs[k][:, c], in0=xb_tiles[k][:, c],
                                     in1=R[:, c])
                continue
            v = workp.tile([128, 1024], BF16, tag="uw", name=f"v_{name}_{k}_{lo}")
            nc.vector.tensor_sub(out=v[:, 0:width], in0=xb_tiles[k][:, c], in1=M[:, c])
            nc.vector.tensor_mul(out=outs[k][:, c], in0=v[:, 0:width], in1=R[:, c])

    def ln_apply(xb_tiles, R, M, out_pool, T, name, out_tag=None, mul_only=False):
        outs = [out_pool.tile([128, T], BF16, tag=(out_tag or "x2n"),
                              name=f"{name}_n_{k}") for k in range(NCH)]
        for lo in range(0, T, 1024):
            ln_apply_chunk(xb_tiles, R, M, outs, name, lo, min(1024, T - lo),
                           mul_only=mul_only)
        return outs

    def load_and_cast(param, T, name):
        """DMA fp32 feature-major input, casting to bf16 in the SWDGE DMA."""
        xb = []
        for k in range(NCH):
            t = xbp.tile([128, T], BF16, tag="xb", name=f"xb_{name}_{k}")
            nc.gpsimd.dma_start(out=t, in_=param[ts(k, 128), :])
            xb.append(t)
        return xb

    def proj(dst, wname, bname, src_tiles, m, T, src_ap_fn=None, nq_range=None,
             rank1=None):
        """dst [128, T] bf16 = (src @ W[:, m-block] + b) feature-major.
        src_ap_fn(tile, nq) may supply a (strided) token AP per 512-chunk.
        PSUM eviction + bias add on the Pool engine (ACT is the attention
        bottleneck, Pool has slack)."""
        w = min(512, T)
        for nq in (range(T // w) if nq_range is None else nq_range):
            ps = pmm.tile([128, 512], F32, tag="mm", name=f"ps_{wname}_{m}_{nq}")
            for k in range(NCH):
                rhs = (src_ap_fn(src_tiles[k], nq) if src_ap_fn
                       else src_tiles[k][:, ts(nq, w)])
                nc.tensor.matmul(ps[:, 0:w], lhsT=wb[wname][k][:, ts(m, 128)], rhs=rhs,
                                 start=(k == 0), stop=(rank1 is None and k == NCH - 1))
            if rank1 is not None:
                row, mr = rank1
                nc.tensor.matmul(ps[:, 0:w], lhsT=row[0:1, ts(m, 128)],
                                 rhs=mr[0:1, ts(nq, w)], start=False, stop=True)
            nc.scalar.activation(out=dst[:, ts(nq, w)], in_=ps[:, 0:w],
                                 func=AF.Identity, bias=vcol(bname, m))

    # token-major V tiles: [128 tokens, n_chunks, 2 heads, 65]
    # (65th col = 1.0 for the softmax denominator row of the AV matmul)
    # PSUM eviction + bias add runs on the idle Pool engine.
    def mk_vt(n_chunks, group):
        vt = vtp.tile([128, n_chunks, 2, 65], BF16, tag=f"vt{group}",
                      name=f"vt_{group}")
        nc.vector.memset(vt[:, :, :, 64:65], 1.0)
        return vt

    def fill_vt(vt, src_tiles, tok_ap_fn, group, chunks, mr=None):
        for c in chunks:
            ps = pmm.tile([128, 128], F32, tag="mm", name=f"psv_{group}_{c}")
            for k in range(NCH):
                nc.tensor.matmul(ps, lhsT=tok_ap_fn(src_tiles[k], c),
                                 rhs=wb["Wv"][k][:, ts(group, 128)],
                                 start=(k == 0), stop=(mr is None and k == NCH - 1))
            if mr is not None:
                nc.tensor.matmul(ps, lhsT=mr[0:1, ts(c, 128)],
                                 rhs=wrow["wvbar"][0:1, ts(group, 128)],
                                 start=False, stop=True)
            nc.vector.scalar_tensor_tensor(
                out=vt[:, c, :, 0:64],
                in0=ps[:].rearrange("p (h d) -> p h d", h=2),
                scalar=1.0,
                in1=bvb[group][:].rearrange("p (h d) -> p h d", h=2),
                op0=OP.bypass, op1=OP.add)

    def make_vt(src_tiles, tok_ap_fn, n_chunks, group, mr=None):
        vt = mk_vt(n_chunks, group)
        fill_vt(vt, src_tiles, tok_ap_fn, group, range(n_chunks), mr=mr)
        return vt

    # ---------------- attention inner ----------------
    def do_attn(name, h, q_ap, o_ap, key_ap_fn, vt_fn, nq):
        kc_per_big = 1024 // nq
        ebs = []
        for big in range(8 // kc_per_big):
            sb = psb.tile([128, 1024], F32, tag="sbig", name=f"sb_{name}_{big}")
            for j in range(kc_per_big):
                kc = big * kc_per_big + j
                nc.tensor.matmul(sb[:, ts(j, nq)], lhsT=key_ap_fn(kc, h), rhs=q_ap,
                                 start=True, stop=True)
            eb = ebp.tile([128, 1024], BF16, tag="eb", name=f"eb_{name}_{big}")
            nc.scalar.activation(out=eb, in_=sb, func=AF.Exp, scale=SCALE)
            ebs.append(eb)
        ops = pmm.tile([128, 512], F32, tag="mm", name=f"ops_{name}")
        for kc in range(8):
            big, j = divmod(kc, kc_per_big)
            nc.tensor.matmul(ops[0:65, 0:nq], lhsT=vt_fn(kc)[:, h, 0:65],
                             rhs=ebs[big][:, ts(j, nq)],
                             start=(kc == 0), stop=(kc == 7))
        rc = smp.tile([1, 512], BF16, tag="rc", name=f"rc_{name}")
        with nc.allow_low_precision(reason="softmax denom reciprocal in bf16"):
            nc.vector.reciprocal(out=rc[:, 0:nq], in_=ops[64:65, 0:nq])
        # broadcast 1/denom across the 64 head dims on the idle Pool engine
        rb = smp.tile([64, 512], BF16, tag="rb", name=f"rb_{name}")
        nc.gpsimd.partition_broadcast(rb[:, 0:nq], rc[0:1, 0:nq], 64)
        nc.vector.tensor_mul(out=o_ap, in0=ops[0:64, 0:nq], in1=rb[:, 0:nq])

    hs = lambda h: slice(h * 64, (h + 1) * 64)

    def dil_ap(tile_, r, off):
        return tile_[:, :].rearrange("p (t r) -> p t r", r=r)[:, :, off]

    # -------- phase 1: LN1 half 0 -> seg-0 Q/K/V -> seg-0 attention --------
    # Getting attention (ACT exps + PE) started ~15us in, overlapped with the
    # DVE-heavy remainder of the LN1 frontend.
    xb_loc = load_and_cast(P["x2T"], T_LOC, "loc")
    R_loc, M_loc = ln_alloc("loc")
    ln_stats_chunk(xb_loc, 0, R_loc, M_loc, "loc")
    ln_stats_chunk(xb_loc, 1, R_loc, M_loc, "loc")
    ln_stats_chunk(xb_loc, 2, R_loc, M_loc, "loc", sq_act=False)
    ln_stats_chunk(xb_loc, 3, R_loc, M_loc, "loc", sq_act=False)
    x2n = [x2np.tile([128, T_LOC], BF16, tag="x2n", name=f"loc_n_{k}")
           for k in range(NCH)]
    ln_apply_chunk(xb_loc, R_loc, M_loc, x2n, "loc", 0, 1024)

    q_sizes = [T_LOC, 1024, 512, 256]
    QT = [acts.tile([128, q_sizes[g]], BF16, tag="act5", name=f"QT_{g}")
          for g in range(4)]
    load_weight("Wq")
    proj(QT[0], "Wq", "bq", x2n, 0, T_LOC, nq_range=(0, 1))
    load_weight("Wk")
    KT0 = acts.tile([128, T_LOC], BF16, tag="act5", name="KT_0")
    proj(KT0, "Wk", "bk", x2n, 0, T_LOC, nq_range=(0, 1))
    load_weight("Wv")
    vt0 = mk_vt(16, 0)
    fill_vt(vt0, x2n, lambda t, c: t[:, ts(c, 128)], 0, range(8))

    def g0_attn(seg, h, qc):
        do_attn(f"g0_{seg}_{h}_{qc}", h,
                QT[0][hs(h), seg * 1024 + qc * 512: seg * 1024 + (qc + 1) * 512],
                oT[0][hs(h), seg * 1024 + qc * 512: seg * 1024 + (qc + 1) * 512],
                lambda kc, h_: KT0[hs(h_), seg * 1024 + kc * 128: seg * 1024 + (kc + 1) * 128],
                lambda kc: vt0[:, seg * 8 + kc], 512)

    for qc in range(2):
        for h in range(2):
            g0_attn(0, h, qc)

    # -------- phase 2: LN1 half 1 + remaining local Q/K/V --------
    ln_apply_chunk(xb_loc, R_loc, M_loc, x2n, "loc", 1024, 1024)
    proj(QT[0], "Wq", "bq", x2n, 0, T_LOC, nq_range=(2, 3))
    proj(QT[1], "Wq", "bq", x2n, 1, 1024,
         lambda t, nq: dil_ap(t, 2, 1)[:, ts(nq, 512)])
    proj(QT[2], "Wq", "bq", x2n, 2, 512,
         lambda t, nq: dil_ap(t, 4, 2)[:, ts(nq, 512)])
    proj(QT[3], "Wq", "bq", x2n, 3, 256,
         lambda t, nq: dil_ap(t, 8, 3)[:, 0:256])
    proj(KT0, "Wk", "bk", x2n, 0, T_LOC, nq_range=(2, 3))
    KTg1 = acts.tile([128, 1024], BF16, tag="act5", name="KT_1")
    proj(KTg1, "Wk", "bk", x2n, 1, 1024,
         lambda t, nq: dil_ap(t, 2, 1)[:, ts(nq, 512)])
    fill_vt(vt0, x2n, lambda t, c: t[:, ts(c, 128)], 0, range(8, 16))
    vt1 = make_vt(x2n, lambda t, c: dil_ap(t, 2, 1)[:, ts(c, 128)], 8, 1)

    # -------- phase 3: side groups 2/3 (chain + attention) --------
    def side_group(xbg, group, name):
        Rg, Mg = ln_stats(xbg, 1024, name, sq_act=False)
        x2ng = ln_apply(xbg, Rg, Mg, x2np, 1024, name,
                        out_tag="x2n_side")
        ktg = acts.tile([128, 1024], BF16, tag="act5", name=f"KTg_{group}")
        proj(ktg, "Wk", "bk", x2ng, group, 1024)
        vtg = make_vt(x2ng, lambda t, c: t[:, ts(c, 128)], 8, group)
        return ktg, vtg

    xb_g2 = load_and_cast(P["xg2T"], 1024, "g2")
    KTg2, vt2 = side_group(xb_g2, 2, "g2")
    for h in range(2):
        do_attn(f"g2_{h}", h, QT[2][hs(h), 0:512],
                dil_ap(oT[2], 4, 2)[hs(h), 0:512],
                lambda kc, h_: KTg2[hs(h_), ts(kc, 128)],
                lambda kc: vt2[:, kc], 512)

    xb_g3 = load_and_cast(P["xg3T"], 1024, "g3")
    KTg3, vt3 = side_group(xb_g3, 3, "g3")
    for h in range(2):
        do_attn(f"g3_{h}", h, QT[3][hs(h), 0:256],
                dil_ap(oT[3], 8, 3)[hs(h), 0:256],
                lambda kc, h_: KTg3[hs(h_), ts(kc, 128)],
                lambda kc: vt3[:, kc], 256)

    # ------------- phase 4: local attention + LN_attn stats (interleaved) ----
    load_weight("Wo")
    R_a, M_a = ln_alloc("lna")

    def g1_attn(h, qc):
        do_attn(f"g1_{h}_{qc}", h, QT[1][hs(h), ts(qc, 512)],
                dil_ap(oT[1], 2, 1)[hs(h), ts(qc, 512)],
                lambda kc, h_: KTg1[hs(h_), ts(kc, 128)],
                lambda kc: vt1[:, kc], 512)

    # Phase 5 halves are interleaved into local attention: the LN_attn
    # stats/apply and Wo projection for columns [0,1024) run (on PE/DVE)
    # while the second half of attention keeps ACT busy with exps.
    o_n = [acts.tile([128, T_LOC], BF16, tag="act5", name=f"lna_n_{k}")
           for k in range(NCH)]
    y1b = [acts.tile([128, T_LOC], BF16, tag="act5", name=f"y1b_{m}")
           for m in range(4)]
    load_weight("W1")
    R_2, M_2 = ln_alloc("ln2")

    def phase5_half(half):
        ln_apply_chunk(oT, R_a, M_a, o_n, "lna", half * 1024, 1024)
        for nq in (2 * half, 2 * half + 1):
            for m in range(4):
                ps = pmm.tile([128, 512], F32, tag="mm", name=f"pso_{m}_{nq}")
                for k in range(NCH):
                    nc.tensor.matmul(ps, lhsT=wb["Wo"][k][:, ts(m, 128)],
                                     rhs=o_n[k][:, ts(nq, 512)],
                                     start=(k == 0), stop=False)
                x1f = stage.tile([128, 512], BF16, tag="x1stage", name=f"x1f_{m}_{nq}")
                nc.gpsimd.dma_start(out=x1f, in_=P["x1T"][ts(m, 128), ts(nq, 512)])
                nc.tensor.matmul(ps, lhsT=ident, rhs=x1f, start=False, stop=True)
                nc.scalar.activation(out=y1b[m][:, ts(nq, 512)], in_=ps,
                                     func=AF.Identity, bias=vcol("bo", m))
                nc.gpsimd.dma_start(out=P["out"][ts(m, 128), ts(nq, 512)],
                                    in_=y1b[m][:, ts(nq, 512)])
        for tq in (2 * half, 2 * half + 1):
            ln_stats_chunk(y1b, tq, R_2, M_2, "ln2", sq_act=False)

    # ---------------- phase 6: LN2 + FFN + residual, per-half ---------------
    # Per-half tiles live in dedicated pools so their allocation never waits
    # on slots held by still-running attention work (deadlock + overlap).
    def phase6_half(half):
        c = slice(half * 1024, half * 1024 + 1024)
        y1n_h = [ffn1p.tile([128, 1024], BF16, tag="y1n", name=f"ln2n_{half}_{k}")
                 for k in range(NCH)]
        for k in range(NCH):
            v = workp.tile([128, 1024], BF16, tag="uw", name=f"v_ln2_{half}_{k}")
            nc.vector.tensor_sub(out=v, in0=y1b[k][:, c], in1=M_2[:, c])
            nc.vector.tensor_mul(out=y1n_h[k], in0=v, in1=R_2[:, c])
        hb_h = [ffn2p.tile([128, 1024], BF16, tag="hb", name=f"hb_{half}_{m}")
                for m in range(4)]
        load_weight("W2")
        for j in range(2):
            nq = 2 * half + j
            for m in range(4):
                ps = pmm.tile([128, 512], F32, tag="mm", name=f"psh_{m}_{nq}")
                for k in range(NCH):
                    nc.tensor.matmul(ps, lhsT=wb["W1"][k][:, ts(m, 128)],
                                     rhs=y1n_h[k][:, ts(j, 512)],
                                     start=(k == 0), stop=(k == NCH - 1))
                nc.scalar.activation(out=hb_h[m][:, ts(j, 512)], in_=ps, func=AF.Relu,
                                     bias=vcol("b1", m))
        for j in range(2):
            nq = 2 * half + j
            for m in range(4):
                ps = pmm.tile([128, 512], F32, tag="mm", name=f"psy2_{m}_{nq}")
                for k in range(NCH):
                    nc.tensor.matmul(ps, lhsT=wb["W2"][k][:, ts(m, 128)],
                                     rhs=hb_h[k][:, ts(j, 512)],
                                     start=(k == 0), stop=False)
                x2f = stage.tile([128, 512], BF16, tag="x2stage", name=f"x2f_{m}_{nq}")
                nc.gpsimd.dma_start(out=x2f, in_=P["x2T"][ts(m, 128), ts(nq, 512)])
                nc.tensor.matmul(ps, lhsT=ident, rhs=x2f, start=False, stop=True)
                y2f = yp.tile([128, 512], F32, tag="y2f", name=f"y2f_{m}_{nq}")
                nc.scalar.activation(out=y2f, in_=ps, func=AF.Identity,
                                     bias=vcol("b2", m))
                nc.sync.dma_start(out=P["out"][ts(m + 4, 128), ts(nq, 512)], in_=y2f)

    # columns 0:1024 of local attention, then their LN_attn stats + phase 5/6,
    # overlapped with the second half of attention.
    for h in range(2):
        g1_attn(h, 0)
    ln_stats_chunk(oT, 0, R_a, M_a, "lna", sq_act=False)
    ln_stats_chunk(oT, 1, R_a, M_a, "lna", sq_act=False)
    for qc in range(2):
        for h in range(2):
            g0_attn(1, h, qc)
    for h in range(2):
        g1_attn(h, 1)
    ln_stats_chunk(oT, 2, R_a, M_a, "lna", sq_act=False)
    ln_stats_chunk(oT, 3, R_a, M_a, "lna", sq_act=False)
    phase5_half(0)
    phase6_half(0)
    phase5_half(1)
    phase6_half(1)


_ACT_PATCHED = False


def _patch_act_tables():
    """Make every activation func we use resolve to the one table set that
    contains them all (natural_log_exp_and_others), so the act-table-load
    pass emits a single load instead of thrashing Ln<->Exp (~1.3us each).
    Set ids are dict-order — membership is edited, order preserved."""
    global _ACT_PATCHED
    if _ACT_PATCHED:
        return
    _ACT_PATCHED = True
    import concourse.hw_specs as _hw
    _orig = _hw.get_activation_tables
    mine = {AF.Exp, AF.Ln, AF.Relu, AF.Identity, AF.Copy}

    def _patched(arch):
        t = _orig(arch)
        return {name: (s if name == "natural_log_exp_and_others" else s - mine)
                for name, s in t.items()}

    bacc.get_activation_tables = _patched


def build_nc():
    _patch_act_tables()
    nc = bacc.Bacc()
    P = {}
    P["x1T"] = nc.declare_dram_parameter("x1T", [DM, T_LOC], F32, isOutput=False)
    P["x2T"] = nc.declare_dram_parameter("x2T", [DM, T_LOC], F32, isOutput=False)
    P["xg2T"] = nc.declare_dram_parameter("xg2T", [DM, T_G2], F32, isOutput=False)
    P["xg3T"] = nc.declare_dram_parameter("xg3T", [DM, T_G3], F32, isOutput=False)
    for w in WEIGHT_NAMES:
        P[w] = nc.declare_dram_parameter(w, [DM, DM], F32, isOutput=False)
    P["vecsT"] = nc.declare_dram_parameter("vecsT", [DM, NVEC], F32, isOutput=False)
    P["vecs"] = nc.declare_dram_parameter("vecs", [NVEC, DM], F32, isOutput=False)
    P["out"] = nc.declare_dram_parameter("out", [2 * DM, T_LOC], F32, isOutput=True)

    from contextlib import ExitStack
    with tile.TileContext(nc) as tc:
        with ExitStack() as ctx:
            _emit(ctx, tc, P)
    nc.finalize()
    return nc


def make_in_maps(x, Wq, bq, Wk, bk, Wv, bv, ln_attn_g, ln_attn_b, Wo, bo,
                 W1, b1, W2, b2, ln1_g, ln1_b, ln2_g, ln2_b):
    """Shard the full inputs into 8 per-core input maps.

    LN gamma/beta are folded into the consumer projections here (pure host
    numpy): W' = g*W and b' = b + W^T ln_b, so the device LN apply is just
    (x - mean) * rstd."""
    x = np.ascontiguousarray(np.asarray(x, dtype=np.float32))
    f32 = lambda a: np.asarray(a, dtype=np.float32)
    Wq, Wk, Wv, Wo, W1, W2 = (f32(W) for W in (Wq, Wk, Wv, Wo, W1, W2))
    bq, bk, bv, bo, b1, b2 = (f32(b) for b in (bq, bk, bv, bo, b1, b2))
    g1, b1n = f32(ln1_g), f32(ln1_b)
    ga, ban = f32(ln_attn_g), f32(ln_attn_b)
    g2, b2n = f32(ln2_g), f32(ln2_b)
    cc = np.ascontiguousarray
    Ws = {"Wq": cc(g1[:, None] * Wq), "Wk": cc(g1[:, None] * Wk),
          "Wv": cc(g1[:, None] * Wv), "Wo": cc(ga[:, None] * Wo),
          "W1": cc(g2[:, None] * W1), "W2": cc(W2)}
    bq = bq + Wq.T @ b1n
    bk = bk + Wk.T @ b1n
    bv = bv + Wv.T @ b1n
    bo = bo + Wo.T @ ban
    b1 = b1 + W1.T @ b2n
    # negated column sums of the scaled weights, for the rank-1 mean
    # correction (side/LNA applies skip the mean subtraction; the consumer
    # matmul accumulates -wbar (x) (m*r) instead)
    wkbar = -Ws["Wk"].sum(axis=0)
    wvbar = -Ws["Wv"].sum(axis=0)
    wobar = -Ws["Wo"].sum(axis=0)
    vec_vals = [bq, bk, bv, bo, b1, b2, ln1_g, ln1_b, ln_attn_g, ln_attn_b,
                ln2_g, ln2_b, wkbar, wvbar, wobar]
    vecs = np.zeros((NVEC, DM), np.float32)
    for i, v in enumerate(vec_vals):
        vecs[i] = np.asarray(v, dtype=np.float32)
    vecsT = np.ascontiguousarray(vecs.T)

    in_maps = []
    for core in range(8):
        b, qt = divmod(core, 4)
        t0 = qt * T_LOC
        x1 = x[b, t0:t0 + T_LOC, 0:DM]
        x2 = x[b, t0:t0 + T_LOC, DM:2 * DM]
        seg2 = qt // 2
        xg2 = x[b, seg2 * 4096 + 2: (seg2 + 1) * 4096: 4, DM:2 * DM]
        xg3 = x[b, 3::8, DM:2 * DM]
        m = {
            "x1T": np.ascontiguousarray(x1.T),
            "x2T": np.ascontiguousarray(x2.T),
            "xg2T": np.ascontiguousarray(xg2.T),
            "xg3T": np.ascontiguousarray(xg3.T),
            "vecsT": vecsT,
            "vecs": vecs,
        }
        m.update(Ws)
        in_maps.append(m)
    return in_maps


_NC_CACHE = None


def get_nc():
    global _NC_CACHE
    if _NC_CACHE is None:
        _NC_CACHE = build_nc()
    return _NC_CACHE


def _execute(in_maps, trace=False, **kwargs):
    nc = get_nc()
    return run_bass_kernel_spmd(nc, in_maps, core_ids=list(range(8)),
                                trace=trace, **kwargs)


def assemble_output(results):
    out = np.empty((2, 8192, 2 * DM), np.float32)
    for core in range(8):
        b, qt = divmod(core, 4)
        out[b, qt * T_LOC:(qt + 1) * T_LOC, :] = results[core]["out"].T
    return out


def kernel(**inputs):
    in_maps = make_in_maps(**inputs)
    res = _execute(in_maps)
    return assemble_output(res.results)



# revision 60
# speedup vs baseline: 1.1715x; 1.0905x over previous
"""Trainium2 Bass kernel: reversible block with 4-group dilated attention.

Reference computation (per batch b, seq n=8192, d_model=1024, DM=512):
    x1, x2 = split(x, 2, -1)
    y1 = x1 + AttnBlock(LN(x2; ln1)) ; y2 = x2 + FFN(LN(y1; ln2))
    out = concat(y1, y2)
AttnBlock(z) = LN(DilatedAttn(zWq+bq, zWk+bk, zWv+bv); ln_attn) @ Wo + bo
DilatedAttn: heads split into 4 groups of 2; group i attends within segments
of length s_i in (1024,2048,4096,8192) at stride r_i in (1,2,4,8), offset i%r_i.

Sharding: 8 cores = batch(2) x seq-quarter(4).  Each quarter of 2048 tokens is
self-contained for groups 0/1; for groups 2/3 the (strided) key/value source
rows are replicated to each core (host-side slicing), so there is no
cross-core communication.  Everything on-device runs feature-major
([channel, token]) with bf16 matmul operands and fp32 accumulation/residuals.
"""

import numpy as np

import concourse.bass as bass
import concourse.mybir as mybir
import concourse.tile as tile
from concourse import bacc
from concourse.bass_utils import run_bass_kernel_spmd

F32 = mybir.dt.float32
BF16 = mybir.dt.bfloat16
AF = mybir.ActivationFunctionType
OP = mybir.AluOpType

DM = 512              # working width seen by attention/FFN
HD = 64               # head dim
T_LOC = 2048          # tokens per core
T_G2 = 1024           # group-2 replicated key rows (one 4096-segment, stride 4)
T_G3 = 1024           # group-3 replicated key rows (whole seq, stride 8)
NCH = DM // 128       # 4 channel chunks
EPS = 1e-5
SCALE = 1.0 / 8.0     # 1/sqrt(HD)

# order of [512]-vectors packed into the "vecs"/"vecsT" inputs
VEC_NAMES = ["bq", "bk", "bv", "bo", "b1", "b2",
             "ln1_g", "ln1_b", "lna_g", "lna_b", "ln2_g", "ln2_b",
             "wkbar", "wvbar", "wobar"]
NVEC = 16  # padded columns

WEIGHT_NAMES = ["Wq", "Wk", "Wv", "Wo", "W1", "W2"]


def ts(i, size):
    return slice(i * size, (i + 1) * size)


def _emit(ctx, tc, P):
    """Emit the whole per-core program into TileContext tc."""
    nc = tc.nc

    # ---------------- pools ----------------
    # NOTE: within a pool, every distinct `tag` gets its own `bufs` slots
    # sized to the max tile of that tag.  SBUF budget is ~208KB/partition.
    pool = lambda **kw: ctx.enter_context(tc.tile_pool(**kw))
    consts = pool(name="consts", bufs=1)     # ~5KB: vectors, ones, bvb
    wtagp = pool(name="wtagp", bufs=12)      # 12KB: bf16 weight chunks (shared slots)
    stage = pool(name="stage", bufs=2)       # bf16 residual staging
    xbp = pool(name="xbp", bufs=4)           # 16KB: bf16 casts of x2-side inputs
    x2np = pool(name="x2np", bufs=4)         # 16KB: LN1 outputs
    acts = pool(name="acts", bufs=12)        # 48KB: large bf16 activations
    vtp = pool(name="vtp", bufs=1)           # ~10KB: one consolidated V tile per group
    otp = pool(name="otp", bufs=1)           # 16KB: attention output (feature-major)
    sqp = pool(name="sqp", bufs=4)           # 4KB: squared tiles for stats
    rmrp = pool(name="rmrp", bufs=2)         # 16KB: R / MR stat rows (bf16, replicated)
    smp = pool(name="smp", bufs=2)           # ~24KB: small stat tiles
    ebp = pool(name="ebp", bufs=6)           # 12KB: exp(S) tiles
    yp = pool(name="yp", bufs=2)             # 8KB: fp32 output staging
    workp = pool(name="workp", bufs=3)       # 6KB: LN apply temps (shared tag)
    ffn1p = pool(name="ffn1p", bufs=4)       # 8KB: per-half LN2 outputs
    ffn2p = pool(name="ffn2p", bufs=4)       # 8KB: per-half ReLU outputs

    pmm = pool(name="pmm", bufs=4, space="PSUM")    # 4 banks: matmul outs + stats
    psb = pool(name="psb", bufs=2, space="PSUM")    # 4 banks: attention logits
    pst = pmm

    # ---------------- constants ----------------
    ones = consts.tile([128, 128], BF16, name="ones")
    nc.vector.memset(ones, 1.0)
    eps_col = consts.tile([128, 1], F32, name="eps_col")
    nc.vector.memset(eps_col, EPS)
    from concourse.masks import make_identity
    ident = consts.tile([128, 128], BF16, name="ident")
    make_identity(nc, ident[:])

    # weights -> bf16 tiles wb[name][k] : [128, DM].  Loaded lazily so the
    # shared wtag slots can recycle (Wq/Wk/Wv early, Wo/W1/W2 later).
    wb = {}

    def load_weight(w):
        if w in wb:
            return
        wb[w] = []
        for k in range(NCH):
            t = wtagp.tile([128, DM], BF16, tag="wtag", name=f"wb_{w}_{k}")
            nc.gpsimd.dma_start(out=t, in_=P[w][ts(k, 128), :])  # casting DMA
            wb[w].append(t)

    # per-channel vectors, channel-major: vst[k] = [128, NVEC]
    vst = []
    for k in range(NCH):
        t = consts.tile([128, NVEC], F32, name=f"vst_{k}")
        nc.sync.dma_start(out=t, in_=P["vecsT"][ts(k, 128), :])
        vst.append(t)

    def vcol(name, k):
        return vst[k][:, VEC_NAMES.index(name):VEC_NAMES.index(name) + 1]

    # bv broadcast across partitions (needed in token-major V tiles):
    # bvb[k] = [128, 128] where free dim = output channels k*128..(k+1)*128
    bvb = []
    bv_i = VEC_NAMES.index("bv")
    for k in range(NCH):
        t = consts.tile([128, 128], F32, name=f"bvb_{k}")
        nc.sync.dma_start(out=t, in_=P["vecs"][bv_i:bv_i + 1, ts(k, 128)].to_broadcast([128, 128]))
        bvb.append(t)

    # attention output assembly tiles, bf16 feature-major [512, T_LOC];
    # groups 1..3 need zeroed gaps — memsets are emitted later (low priority)
    # so they fill DVE slack instead of competing with the LN1 frontend.
    oT = [otp.tile([128, T_LOC], BF16, name=f"oT_{i}", tag=f"oT_{i}")
          for i in range(4)]
    for i in range(1, 4):
        nc.vector.memset(oT[i], 0.0)

    # ---------------- LN helpers (feature-major) ----------------
    # LN output is produced as just (x - m) * R.  gamma is pre-folded into
    # the projection weights on the host (W' = g*W) and beta into the
    # projection biases (b' = b + W^T ln_b), so every consumer matmul sees
    # the full LN for free and the apply is two cheap bf16 TensorTensors.
    def ln_alloc(name):
        R = rmrp.tile([128, T_LOC], BF16, tag="R", name=f"R_{name}")
        M = rmrp.tile([128, T_LOC], BF16, tag="M", name=f"M_{name}")
        return R, M

    def ln_stats_chunk(xb_tiles, tq, R, M, name, sq_act=False):
        """One 512-column chunk of feature-major LN stats.
        R = rstd rows, M = mean rows (both replicated over partitions).
        sq_act: square on ACT (frontend, where DVE is the bottleneck)."""
        s_ps = pmm.tile([128, 512], F32, tag="mm", name=f"sps_{name}_{tq}")
        q_ps = pmm.tile([128, 512], F32, tag="mm", name=f"qps_{name}_{tq}")
        for k in range(NCH):
            xsl = xb_tiles[k][:, ts(tq, 512)]
            sq = sqp.tile([128, 512], BF16, tag="sq", name=f"sq_{name}_{tq}_{k}")
            if sq_act:
                nc.scalar.activation(out=sq, in_=xsl, func=AF.Square)
            else:
                nc.vector.tensor_mul(out=sq, in0=xsl, in1=xsl)
            nc.tensor.matmul(s_ps, lhsT=ones, rhs=xsl, start=(k == 0), stop=(k == NCH - 1))
            nc.tensor.matmul(q_ps, lhsT=ones, rhs=sq, start=(k == 0), stop=(k == NCH - 1))
        m_sl = M[:, ts(tq, 512)]
        nc.vector.tensor_scalar(out=m_sl, in0=s_ps, scalar1=1.0 / DM, scalar2=0.0,
                                op0=OP.mult, op1=OP.add)
        msq = smp.tile([128, 512], BF16, tag="msq", name=f"msq_{name}_{tq}")
        nc.vector.tensor_mul(out=msq, in0=m_sl, in1=m_sl)
        var = smp.tile([128, 512], F32, tag="var", name=f"var_{name}_{tq}")
        nc.vector.scalar_tensor_tensor(out=var, in0=q_ps, scalar=1.0 / DM, in1=msq,
                                       op0=OP.mult, op1=OP.subtract)
        lnv = smp.tile([128, 512], F32, tag="lnv", name=f"lnv_{name}_{tq}")
        nc.scalar.activation(out=lnv, in_=var, func=AF.Ln, bias=eps_col)
        nc.scalar.activation(out=R[:, ts(tq, 512)], in_=lnv, func=AF.Exp, scale=-0.5)

    def ln_stats(xb_tiles, T, name, sq_act=False):
        R, M = ln_alloc(name)
        for tq in range(T // 512):
            ln_stats_chunk(xb_tiles, tq, R, M, name, sq_act=sq_act)
        return R, M

    def ln_apply_chunk(xb_tiles, R, M, outs, name, lo, width, mul_only=False):
        """out = (x - m) * R over columns [lo, lo+width), all NCH chunks.
        mul_only: out = x * R; the mean term is restored by a rank-1
        -wbar (x) (m*r) matmul inside the consumer projection."""
        c = slice(lo, lo + width)
        for k in range(NCH):
            if mul_only:
                nc.vector.tensor_mul(out=outs[k][:, c], in0=xb_tiles[k][:, c],
                                     in1=R[:, c])
                continue
            v = workp.tile([128, 1024], BF16, tag="uw", name=f"v_{name}_{k}_{lo}")
            nc.vector.tensor_sub(out=v[:, 0:width], in0=xb_tiles[k][:, c], in1=M[:, c])
            nc.vector.tensor_mul(out=outs[k][:, c], in0=v[:, 0:width], in1=R[:, c])

    def ln_apply(xb_tiles, R, M, out_pool, T, name, out_tag=None, mul_only=False):
        outs = [out_pool.tile([128, T], BF16, tag=(out_tag or "x2n"),
                              name=f"{name}_n_{k}") for k in range(NCH)]
        for lo in range(0, T, 1024):
            ln_apply_chunk(xb_tiles, R, M, outs, name, lo, min(1024, T - lo),
                           mul_only=mul_only)
        return outs

    def load_and_cast(param, T, name):
        """DMA fp32 feature-major input, casting to bf16 in the SWDGE DMA."""
        xb = []
        for k in range(NCH):
            t = xbp.tile([128, T], BF16, tag="xb", name=f"xb_{name}_{k}")
            nc.gpsimd.dma_start(out=t, in_=param[ts(k, 128), :])
            xb.append(t)
        return xb

    def proj(dst, wname, bname, src_tiles, m, T, src_ap_fn=None, nq_range=None,
             rank1=None):
        """dst [128, T] bf16 = (src @ W[:, m-block] + b) feature-major.
        src_ap_fn(tile, nq) may supply a (strided) token AP per 512-chunk.
        PSUM eviction + bias add on the Pool engine (ACT is the attention
        bottleneck, Pool has slack)."""
        w = min(512, T)
        for nq in (range(T // w) if nq_range is None else nq_range):
            ps = pmm.tile([128, 512], F32, tag="mm", name=f"ps_{wname}_{m}_{nq}")
            for k in range(NCH):
                rhs = (src_ap_fn(src_tiles[k], nq) if src_ap_fn
                       else src_tiles[k][:, ts(nq, w)])
                nc.tensor.matmul(ps[:, 0:w], lhsT=wb[wname][k][:, ts(m, 128)], rhs=rhs,
                                 start=(k == 0), stop=(rank1 is None and k == NCH - 1))
            if rank1 is not None:
                row, mr = rank1
                nc.tensor.matmul(ps[:, 0:w], lhsT=row[0:1, ts(m, 128)],
                                 rhs=mr[0:1, ts(nq, w)], start=False, stop=True)
            nc.scalar.activation(out=dst[:, ts(nq, w)], in_=ps[:, 0:w],
                                 func=AF.Identity, bias=vcol(bname, m))

    # token-major V tiles: [128 tokens, n_chunks, 2 heads, 65]
    # (65th col = 1.0 for the softmax denominator row of the AV matmul)
    # PSUM eviction + bias add runs on the idle Pool engine.
    def mk_vt(n_chunks, group):
        vt = vtp.tile([128, n_chunks, 2, 65], BF16, tag=f"vt{group}",
                      name=f"vt_{group}")
        nc.vector.memset(vt[:, :, :, 64:65], 1.0)
        return vt

    def fill_vt(vt, src_tiles, tok_ap_fn, group, chunks, mr=None):
        for c in chunks:
            ps = pmm.tile([128, 128], F32, tag="mm", name=f"psv_{group}_{c}")
            for k in range(NCH):
                nc.tensor.matmul(ps, lhsT=tok_ap_fn(src_tiles[k], c),
                                 rhs=wb["Wv"][k][:, ts(group, 128)],
                                 start=(k == 0), stop=(mr is None and k == NCH - 1))
            if mr is not None:
                nc.tensor.matmul(ps, lhsT=mr[0:1, ts(c, 128)],
                                 rhs=wrow["wvbar"][0:1, ts(group, 128)],
                                 start=False, stop=True)
            nc.vector.scalar_tensor_tensor(
                out=vt[:, c, :, 0:64],
                in0=ps[:].rearrange("p (h d) -> p h d", h=2),
                scalar=1.0,
                in1=bvb[group][:].rearrange("p (h d) -> p h d", h=2),
                op0=OP.bypass, op1=OP.add)

    def make_vt(src_tiles, tok_ap_fn, n_chunks, group, mr=None):
        vt = mk_vt(n_chunks, group)
        fill_vt(vt, src_tiles, tok_ap_fn, group, range(n_chunks), mr=mr)
        return vt

    # ---------------- attention inner ----------------
    def do_attn(name, h, q_ap, o_ap, key_ap_fn, vt_fn, nq):
        kc_per_big = 1024 // nq
        ebs = []
        for big in range(8 // kc_per_big):
            sb = psb.tile([128, 1024], F32, tag="sbig", name=f"sb_{name}_{big}")
            for j in range(kc_per_big):
                kc = big * kc_per_big + j
                nc.tensor.matmul(sb[:, ts(j, nq)], lhsT=key_ap_fn(kc, h), rhs=q_ap,
                                 start=True, stop=True)
            eb = ebp.tile([128, 1024], BF16, tag="eb", name=f"eb_{name}_{big}")
            nc.scalar.activation(out=eb, in_=sb, func=AF.Exp, scale=SCALE)
            ebs.append(eb)
        ops = pmm.tile([128, 512], F32, tag="mm", name=f"ops_{name}")
        for kc in range(8):
            big, j = divmod(kc, kc_per_big)
            nc.tensor.matmul(ops[0:65, 0:nq], lhsT=vt_fn(kc)[:, h, 0:65],
                             rhs=ebs[big][:, ts(j, nq)],
                             start=(kc == 0), stop=(kc == 7))
        rc = smp.tile([1, 512], BF16, tag="rc", name=f"rc_{name}")
        with nc.allow_low_precision(reason="softmax denom reciprocal in bf16"):
            nc.vector.reciprocal(out=rc[:, 0:nq], in_=ops[64:65, 0:nq])
        # broadcast 1/denom across the 64 head dims on the idle Pool engine
        rb = smp.tile([64, 512], BF16, tag="rb", name=f"rb_{name}")
        nc.gpsimd.partition_broadcast(rb[:, 0:nq], rc[0:1, 0:nq], 64)
        nc.vector.tensor_mul(out=o_ap, in0=ops[0:64, 0:nq], in1=rb[:, 0:nq])

    hs = lambda h: slice(h * 64, (h + 1) * 64)

    def dil_ap(tile_, r, off):
        return tile_[:, :].rearrange("p (t r) -> p t r", r=r)[:, :, off]

    # -------- phase 1: LN1 half 0 -> seg-0 Q/K/V -> seg-0 attention --------
    # Getting attention (ACT exps + PE) started ~15us in, overlapped with the
    # DVE-heavy remainder of the LN1 frontend.
    xb_loc = load_and_cast(P["x2T"], T_LOC, "loc")
    R_loc, M_loc = ln_alloc("loc")
    ln_stats_chunk(xb_loc, 0, R_loc, M_loc, "loc")
    ln_stats_chunk(xb_loc, 1, R_loc, M_loc, "loc")
    ln_stats_chunk(xb_loc, 2, R_loc, M_loc, "loc", sq_act=False)
    ln_stats_chunk(xb_loc, 3, R_loc, M_loc, "loc", sq_act=False)
    x2n = [x2np.tile([128, T_LOC], BF16, tag="x2n", name=f"loc_n_{k}")
           for k in range(NCH)]
    ln_apply_chunk(xb_loc, R_loc, M_loc, x2n, "loc", 0, 1024)

    q_sizes = [T_LOC, 1024, 512, 256]
    QT = [acts.tile([128, q_sizes[g]], BF16, tag="act5", name=f"QT_{g}")
          for g in range(4)]
    load_weight("Wq")
    proj(QT[0], "Wq", "bq", x2n, 0, T_LOC, nq_range=(0, 1))
    load_weight("Wk")
    KT0 = acts.tile([128, T_LOC], BF16, tag="act5", name="KT_0")
    proj(KT0, "Wk", "bk", x2n, 0, T_LOC, nq_range=(0, 1))
    load_weight("Wv")
    vt0 = mk_vt(16, 0)
    fill_vt(vt0, x2n, lambda t, c: t[:, ts(c, 128)], 0, range(8))

    def g0_attn(seg, h, qc):
        do_attn(f"g0_{seg}_{h}_{qc}", h,
                QT[0][hs(h), seg * 1024 + qc * 512: seg * 1024 + (qc + 1) * 512],
                oT[0][hs(h), seg * 1024 + qc * 512: seg * 1024 + (qc + 1) * 512],
                lambda kc, h_: KT0[hs(h_), seg * 1024 + kc * 128: seg * 1024 + (kc + 1) * 128],
                lambda kc: vt0[:, seg * 8 + kc], 512)

    # apply half-1 now: releases the xb_loc tiles so the side-group loads
    # (same xbp slots) can start ~10us earlier
    ln_apply_chunk(xb_loc, R_loc, M_loc, x2n, "loc", 1024, 1024)

    for qc in range(2):
        for h in range(2):
            g0_attn(0, h, qc)

    # -------- phase 2: remaining local Q/K/V --------
    proj(QT[0], "Wq", "bq", x2n, 0, T_LOC, nq_range=(2, 3))
    proj(QT[1], "Wq", "bq", x2n, 1, 1024,
         lambda t, nq: dil_ap(t, 2, 1)[:, ts(nq, 512)])
    proj(QT[2], "Wq", "bq", x2n, 2, 512,
         lambda t, nq: dil_ap(t, 4, 2)[:, ts(nq, 512)])
    proj(QT[3], "Wq", "bq", x2n, 3, 256,
         lambda t, nq: dil_ap(t, 8, 3)[:, 0:256])
    proj(KT0, "Wk", "bk", x2n, 0, T_LOC, nq_range=(2, 3))
    KTg1 = acts.tile([128, 1024], BF16, tag="act5", name="KT_1")
    proj(KTg1, "Wk", "bk", x2n, 1, 1024,
         lambda t, nq: dil_ap(t, 2, 1)[:, ts(nq, 512)])
    fill_vt(vt0, x2n, lambda t, c: t[:, ts(c, 128)], 0, range(8, 16))
    vt1 = make_vt(x2n, lambda t, c: dil_ap(t, 2, 1)[:, ts(c, 128)], 8, 1)

    # -------- phase 3: side groups 2/3 (chain + attention) --------
    def side_group(xbg, group, name):
        Rg, Mg = ln_stats(xbg, 1024, name, sq_act=False)
        x2ng = ln_apply(xbg, Rg, Mg, x2np, 1024, name,
                        out_tag="x2n_side")
        ktg = acts.tile([128, 1024], BF16, tag="act5", name=f"KTg_{group}")
        proj(ktg, "Wk", "bk", x2ng, group, 1024)
        vtg = make_vt(x2ng, lambda t, c: t[:, ts(c, 128)], 8, group)
        return ktg, vtg

    xb_g2 = load_and_cast(P["xg2T"], 1024, "g2")
    KTg2, vt2 = side_group(xb_g2, 2, "g2")
    for h in range(2):
        do_attn(f"g2_{h}", h, QT[2][hs(h), 0:512],
                dil_ap(oT[2], 4, 2)[hs(h), 0:512],
                lambda kc, h_: KTg2[hs(h_), ts(kc, 128)],
                lambda kc: vt2[:, kc], 512)

    xb_g3 = load_and_cast(P["xg3T"], 1024, "g3")
    KTg3, vt3 = side_group(xb_g3, 3, "g3")
    for h in range(2):
        do_attn(f"g3_{h}", h, QT[3][hs(h), 0:256],
                dil_ap(oT[3], 8, 3)[hs(h), 0:256],
                lambda kc, h_: KTg3[hs(h_), ts(kc, 128)],
                lambda kc: vt3[:, kc], 256)

    # ------------- phase 4: local attention + LN_attn stats (interleaved) ----
    load_weight("Wo")
    R_a, M_a = ln_alloc("lna")

    def g1_attn(h, qc):
        do_attn(f"g1_{h}_{qc}", h, QT[1][hs(h), ts(qc, 512)],
                dil_ap(oT[1], 2, 1)[hs(h), ts(qc, 512)],
                lambda kc, h_: KTg1[hs(h_), ts(kc, 128)],
                lambda kc: vt1[:, kc], 512)

    # Phase 5 halves are interleaved into local attention: the LN_attn
    # stats/apply and Wo projection for columns [0,1024) run (on PE/DVE)
    # while the second half of attention keeps ACT busy with exps.
    o_n = [acts.tile([128, T_LOC], BF16, tag="act5", name=f"lna_n_{k}")
           for k in range(NCH)]
    y1b = [acts.tile([128, T_LOC], BF16, tag="act5", name=f"y1b_{m}")
           for m in range(4)]
    load_weight("W1")
    R_2, M_2 = ln_alloc("ln2")

    def phase5_half(half):
        ln_apply_chunk(oT, R_a, M_a, o_n, "lna", half * 1024, 1024)
        for nq in (2 * half, 2 * half + 1):
            for m in range(4):
                ps = pmm.tile([128, 512], F32, tag="mm", name=f"pso_{m}_{nq}")
                for k in range(NCH):
                    nc.tensor.matmul(ps, lhsT=wb["Wo"][k][:, ts(m, 128)],
                                     rhs=o_n[k][:, ts(nq, 512)],
                                     start=(k == 0), stop=False)
                x1f = stage.tile([128, 512], BF16, tag="x1stage", name=f"x1f_{m}_{nq}")
                nc.gpsimd.dma_start(out=x1f, in_=P["x1T"][ts(m, 128), ts(nq, 512)])
                nc.tensor.matmul(ps, lhsT=ident, rhs=x1f, start=False, stop=True)
                nc.scalar.activation(out=y1b[m][:, ts(nq, 512)], in_=ps,
                                     func=AF.Identity, bias=vcol("bo", m))
                nc.gpsimd.dma_start(out=P["out"][ts(m, 128), ts(nq, 512)],
                                    in_=y1b[m][:, ts(nq, 512)])
        for tq in (2 * half, 2 * half + 1):
            ln_stats_chunk(y1b, tq, R_2, M_2, "ln2", sq_act=False)

    # ---------------- phase 6: LN2 + FFN + residual, per-half ---------------
    # Per-half tiles live in dedicated pools so their allocation never waits
    # on slots held by still-running attention work (deadlock + overlap).
    def phase6_half(half):
        c = slice(half * 1024, half * 1024 + 1024)
        y1n_h = [ffn1p.tile([128, 1024], BF16, tag="y1n", name=f"ln2n_{half}_{k}")
                 for k in range(NCH)]
        for k in range(NCH):
            v = workp.tile([128, 1024], BF16, tag="uw", name=f"v_ln2_{half}_{k}")
            nc.vector.tensor_sub(out=v, in0=y1b[k][:, c], in1=M_2[:, c])
            nc.vector.tensor_mul(out=y1n_h[k], in0=v, in1=R_2[:, c])
        hb_h = [ffn2p.tile([128, 1024], BF16, tag="hb", name=f"hb_{half}_{m}")
                for m in range(4)]
        load_weight("W2")
        for j in range(2):
            nq = 2 * half + j
            for m in range(4):
                ps = pmm.tile([128, 512], F32, tag="mm", name=f"psh_{m}_{nq}")
                for k in range(NCH):
                    nc.tensor.matmul(ps, lhsT=wb["W1"][k][:, ts(m, 128)],
                                     rhs=y1n_h[k][:, ts(j, 512)],
                                     start=(k == 0), stop=(k == NCH - 1))
                nc.scalar.activation(out=hb_h[m][:, ts(j, 512)], in_=ps, func=AF.Relu,
                                     bias=vcol("b1", m))
        for j in range(2):
            nq = 2 * half + j
            for m in range(4):
                ps = pmm.tile([128, 512], F32, tag="mm", name=f"psy2_{m}_{nq}")
                for k in range(NCH):
                    nc.tensor.matmul(ps, lhsT=wb["W2"][k][:, ts(m, 128)],
                                     rhs=hb_h[k][:, ts(j, 512)],
                                     start=(k == 0), stop=False)
                x2f = stage.tile([128, 512], BF16, tag="x2stage", name=f"x2f_{m}_{nq}")
                nc.gpsimd.dma_start(out=x2f, in_=P["x2T"][ts(m, 128), ts(nq, 512)])
                nc.tensor.matmul(ps, lhsT=ident, rhs=x2f, start=False, stop=True)
                y2f = yp.tile([128, 512], F32, tag="y2f", name=f"y2f_{m}_{nq}")
                nc.scalar.activation(out=y2f, in_=ps, func=AF.Identity,
                                     bias=vcol("b2", m))
                nc.sync.dma_start(out=P["out"][ts(m + 4, 128), ts(nq, 512)], in_=y2f)

    # columns 0:1024 of local attention, then their LN_attn stats + phase 5/6,
    # overlapped with the second half of attention.
    for h in range(2):
        g1_attn(h, 0)
    ln_stats_chunk(oT, 0, R_a, M_a, "lna", sq_act=False)
    ln_stats_chunk(oT, 1, R_a, M_a, "lna", sq_act=False)
    for qc in range(2):
        for h in range(2):
            g0_attn(1, h, qc)
    for h in range(2):
        g1_attn(h, 1)
    ln_stats_chunk(oT, 2, R_a, M_a, "lna", sq_act=False)
    ln_stats_chunk(oT, 3, R_a, M_a, "lna", sq_act=False)
    phase5_half(0)
    phase6_half(0)
    phase5_half(1)
    phase6_half(1)


_ACT_PATCHED = False


def _patch_act_tables():
    """Make every activation func we use resolve to the one table set that
    contains them all (natural_log_exp_and_others), so the act-table-load
    pass emits a single load instead of thrashing Ln<->Exp (~1.3us each).
    Set ids are dict-order — membership is edited, order preserved."""
    global _ACT_PATCHED
    if _ACT_PATCHED:
        return
    _ACT_PATCHED = True
    import concourse.hw_specs as _hw
    _orig = _hw.get_activation_tables
    mine = {AF.Exp, AF.Ln, AF.Relu, AF.Identity, AF.Copy}

    def _patched(arch):
        t = _orig(arch)
        return {name: (s if name == "natural_log_exp_and_others" else s - mine)
                for name, s in t.items()}

    bacc.get_activation_tables = _patched


def build_nc():
    _patch_act_tables()
    nc = bacc.Bacc()
    P = {}
    P["x1T"] = nc.declare_dram_parameter("x1T", [DM, T_LOC], F32, isOutput=False)
    P["x2T"] = nc.declare_dram_parameter("x2T", [DM, T_LOC], F32, isOutput=False)
    P["xg2T"] = nc.declare_dram_parameter("xg2T", [DM, T_G2], F32, isOutput=False)
    P["xg3T"] = nc.declare_dram_parameter("xg3T", [DM, T_G3], F32, isOutput=False)
    for w in WEIGHT_NAMES:
        P[w] = nc.declare_dram_parameter(w, [DM, DM], F32, isOutput=False)
    P["vecsT"] = nc.declare_dram_parameter("vecsT", [DM, NVEC], F32, isOutput=False)
    P["vecs"] = nc.declare_dram_parameter("vecs", [NVEC, DM], F32, isOutput=False)
    P["out"] = nc.declare_dram_parameter("out", [2 * DM, T_LOC], F32, isOutput=True)

    from contextlib import ExitStack
    with tile.TileContext(nc) as tc:
        with ExitStack() as ctx:
            _emit(ctx, tc, P)
    nc.finalize()
    return nc


def make_in_maps(x, Wq, bq, Wk, bk, Wv, bv, ln_attn_g, ln_attn_b, Wo, bo,
                 W1, b1, W2, b2, ln1_g, ln1_b, ln2_g, ln2_b):
    """Shard the full inputs into 8 per-core input maps.

    LN gamma/beta are folded into the consumer projections here (pure host
    numpy): W' = g*W and b' = b + W^T ln_b, so the device LN apply is just
    (x - mean) * rstd."""
    x = np.ascontiguousarray(np.asarray(x, dtype=np.float32))
    f32 = lambda a: np.asarray(a, dtype=np.float32)
    Wq, Wk, Wv, Wo, W1, W2 = (f32(W) for W in (Wq, Wk, Wv, Wo, W1, W2))
    bq, bk, bv, bo, b1, b2 = (f32(b) for b in (bq, bk, bv, bo, b1, b2))
    g1, b1n = f32(ln1_g), f32(ln1_b)
    ga, ban = f32(ln_attn_g), f32(ln_attn_b)
    g2, b2n = f32(ln2_g), f32(ln2_b)
    cc = np.ascontiguousarray
    Ws = {"Wq": cc(g1[:, None] * Wq), "Wk": cc(g1[:, None] * Wk),
          "Wv": cc(g1[:, None] * Wv), "Wo": cc(ga[:, None] * Wo),
          "W1": cc(g2[:, None] * W1), "W2": cc(W2)}
    bq = bq + Wq.T @ b1n
    bk = bk + Wk.T @ b1n
    bv = bv + Wv.T @ b1n
    bo = bo + Wo.T @ ban
    b1 = b1 + W1.T @ b2n
    # negated column sums of the scaled weights, for the rank-1 mean
    # correction (side/LNA applies skip the mean subtraction; the consumer
    # matmul accumulates -wbar (x) (m*r) instead)
    wkbar = -Ws["Wk"].sum(axis=0)
    wvbar = -Ws["Wv"].sum(axis=0)
    wobar = -Ws["Wo"].sum(axis=0)
    vec_vals = [bq, bk, bv, bo, b1, b2, ln1_g, ln1_b, ln_attn_g, ln_attn_b,
                ln2_g, ln2_b, wkbar, wvbar, wobar]
    vecs = np.zeros((NVEC, DM), np.float32)
    for i, v in enumerate(vec_vals):
        vecs[i] = np.asarray(v, dtype=np.float32)
    vecsT = np.ascontiguousarray(vecs.T)

    in_maps = []
    for core in range(8):
        b, qt = divmod(core, 4)
        t0 = qt * T_LOC
        x1 = x[b, t0:t0 + T_LOC, 0:DM]
        x2 = x[b, t0:t0 + T_LOC, DM:2 * DM]
        seg2 = qt // 2
        xg2 = x[b, seg2 * 4096 + 2: (seg2 + 1) * 4096: 4, DM:2 * DM]
        xg3 = x[b, 3::8, DM:2 * DM]
        m = {
            "x1T": np.ascontiguousarray(x1.T),
            "x2T": np.ascontiguousarray(x2.T),
            "xg2T": np.ascontiguousarray(xg2.T),
            "xg3T": np.ascontiguousarray(xg3.T),
            "vecsT": vecsT,
            "vecs": vecs,
        }
        m.update(Ws)
        in_maps.append(m)
    return in_maps


_NC_CACHE = None


def get_nc():
    global _NC_CACHE
    if _NC_CACHE is None:
        _NC_CACHE = build_nc()
    return _NC_CACHE


def _execute(in_maps, trace=False, **kwargs):
    nc = get_nc()
    return run_bass_kernel_spmd(nc, in_maps, core_ids=list(range(8)),
                                trace=trace, **kwargs)


def assemble_output(results):
    out = np.empty((2, 8192, 2 * DM), np.float32)
    for core in range(8):
        b, qt = divmod(core, 4)
        out[b, qt * T_LOC:(qt + 1) * T_LOC, :] = results[core]["out"].T
    return out


def kernel(**inputs):
    in_maps = make_in_maps(**inputs)
    res = _execute(in_maps)
    return assemble_output(res.results)

